# revision 8
# baseline (speedup 1.0000x reference)
"""CRF log-likelihood kernel for Trainium2 (Bass/Tile), 8-core data parallel.

out[b] = gold_path_score(b) - logZ(b)

logZ via chunked cold-start forward chains (exp domain).  The transfer
operator D_{el_t} Wf^T is strictly positive, so it contracts directions at
~0.3/step (Birkhoff): a chain started from the uniform vector forgets its
start after ~10 steps.  Split t = 1..T into C chunks of S steps; chain c
starts cold at t = c*S and runs BURN warm-up ticks + S valid ticks, all C
chains advancing in lockstep columns of the same matmul.  Serial depth drops
from T to SP = BURN + S ticks.

Scale recovery: chain c's state is gamma_c * alpha_t (unknown scalar
gamma_c).  At boundary time c*S + SP - 1 both chain c (final tick) and chain
c+1 (tick BURN) hold the same alpha direction, so the ratio of their Wf
column sums gives gamma_{c+1}/gamma_c exactly.  Sink rows capture
sum(alpha_{len-1}) at t == len per sequence (el32 gating), persist to the
chain end, and are read from the final tick's PSUM.  Host stitches:
logZ = log(sink_j) - log(gamma_j) + CSHIFT*len,  j = chunk containing len.

Per-core layout (128 sequences): partitions 0..95 = 3 label groups x 32,
96..98 = per-group sink rows; psum rows 99..101 = per-group column sums.
Columns: chain c occupies cols [43c, 43c+43); within a column, the 3 label
groups hold 3 different sequences (43+43+42+pad = 128).  Two column groups
of 10 chains each give two independent MM->TT streams that overlap on
PE/DVE.  Emissions exp'd on host, shipped bf16 (4x less DMA than f32
logits; el rounding is ~0.4% -> ~0.1 absolute logZ noise over 512 steps).
"""

import numpy as np
import ml_dtypes

B, T, L = 1024, 512, 32
NCORES = 8
BPC = B // NCORES        # 128 sequences per core
NCOL = 43                # columns per chain (3 label groups: 43+43+42+pad)
NACT = 96                # active label partitions
NPART = 99               # + 3 sink rows
MOUT = 102               # + 3 colsum rows
CSHIFT = 4.5
C = 20                   # chains (chunks)
BURN = 12                # cold-start warm-up ticks
S = (T - BURN) // C      # valid ticks per chain (25)
assert BURN + C * S == T
SP = BURN + S            # ticks per chain (37)
COLS = NCOL * C          # 860 total columns
GSPLIT = 10 * NCOL       # group A/B: 10 chains (430 cols) each

_prog_cache = {}
last_result = None       # BassKernelResults of the most recent run (for test.py)


def _build_program():
    import concourse.bacc as bacc
    import concourse.tile as tile
    from concourse import mybir

    f32 = mybir.dt.float32
    bf16 = mybir.dt.bfloat16
    AF = mybir.ActivationFunctionType

    nc = bacc.Bacc("TRN2", target_bir_lowering=False, debug=False, num_devices=NCORES)
    el_d = nc.dram_tensor("el", [NPART, SP, COLS], bf16, kind="ExternalInput")
    u0_d = nc.dram_tensor("u0", [NPART, NCOL], bf16, kind="ExternalInput")
    wf_d = nc.dram_tensor("wf", [NPART, MOUT], bf16, kind="ExternalInput")
    snapE_d = nc.dram_tensor("snapE", [6, COLS], f32, kind="ExternalOutput")
    snapF_d = nc.dram_tensor("snapF", [6, COLS], f32, kind="ExternalOutput")

    with tile.TileContext(nc) as tc:
        with (
            tc.tile_pool(name="consts", bufs=1) as consts,
            tc.tile_pool(name="elp", bufs=1) as elp,
            tc.tile_pool(name="uA", bufs=3) as uApool,
            tc.tile_pool(name="uB", bufs=3) as uBpool,
            tc.tile_pool(name="fin", bufs=1) as fin,
            tc.tile_pool(name="psA", bufs=3, space="PSUM") as psApool,
            tc.tile_pool(name="psB", bufs=3, space="PSUM") as psBpool,
        ):
            wf_sb = consts.tile([NPART, MOUT], bf16)
            u0_sb = consts.tile([NPART, COLS], bf16)
            el_sb = elp.tile([NPART, SP, COLS], bf16)
            finE = fin.tile([6, COLS], f32)
            finF = fin.tile([6, COLS], f32)

            # cold-start state is uniform 1 (sinks 0) everywhere except the
            # chain-0 block, which is DMA'd (= alpha_0); memsets keep the big
            # u0 off the DMA critical path.
            nc.vector.memset(u0_sb[:], 1.0)
            nc.vector.memset(u0_sb[NACT:NPART, :], 0.0)
            # wf + u0 block gate the first MM: sync queue, first.
            nc.sync.dma_start(out=wf_sb[:], in_=wf_d[:])
            nc.sync.dma_start(out=u0_sb[:, 0:NCOL], in_=u0_d[:])
            # el streams on the idle engines' DMA queues (gpsimd/scalar/sync)
            # so chunks transfer concurrently instead of head-of-line
            # blocking on one queue.
            edges = [0, 1, 2, 4, 6, 8, 10, 13, 16, 19, 22, 25, 28, 31, 34, SP]
            qs = [nc.gpsimd, nc.scalar, nc.sync]
            for i, (k0, k1) in enumerate(zip(edges[:-1], edges[1:])):
                qs[i % 3].dma_start(
                    out=el_sb[:, k0:k1, :], in_=el_d[:, k0:k1, :]
                )

            groups = [
                (uApool, psApool, 0, GSPLIT),
                (uBpool, psBpool, GSPLIT, COLS),
            ]
            uprev = [u0_sb[:, c0:c1] for (_, _, c0, c1) in groups]
            for k in range(1, SP + 1):
                for gi, (upool, pspool, c0, c1) in enumerate(groups):
                    ps = pspool.tile([MOUT, c1 - c0], f32, tag=f"ps{gi}")
                    nc.tensor.matmul(ps[:], wf_sb[:], uprev[gi], start=True, stop=True)
                    if k < SP:
                        un = upool.tile([NPART, c1 - c0], bf16, tag=f"u{gi}")
                        nc.vector.tensor_mul(
                            un[:], ps[0:NPART, :], el_sb[:, k - 1, c0:c1]
                        )
                        uprev[gi] = un[:]
                    if k == BURN:
                        nc.scalar.activation(
                            finE[:, c0:c1], ps[NACT:MOUT, :], AF.Copy
                        )
                    if k == SP:
                        nc.scalar.activation(
                            finF[:, c0:c1], ps[NACT:MOUT, :], AF.Copy
                        )
                        (nc.gpsimd if gi == 0 else nc.scalar).dma_start(
                            out=snapF_d[:, c0:c1], in_=finF[:, c0:c1]
                        )
            nc.sync.dma_start(out=snapE_d[:], in_=finE[:])

    nc.compile()
    return nc


def _host_prep(logits, trans, labels, seq_lens):
    logits = np.ascontiguousarray(np.asarray(logits), dtype=np.float32)
    trans = np.asarray(trans, dtype=np.float32)
    labels = np.asarray(labels)
    lens = np.clip(np.asarray(seq_lens), 1, T).astype(np.int64)

    # ---- gold path score (host: index gathers over small inputs) ----
    tmask = np.arange(T)[None, :] < lens[:, None]
    unary = np.take_along_axis(logits, labels[..., None].astype(np.int64), axis=2)[..., 0]
    gp = (unary * tmask).sum(1) + (trans[labels[:, :-1], labels[:, 1:]] * tmask[:, 1:]).sum(1)

    # ---- emissions: exp on host, masked past seq end; slice t=T is
    # capture-only (el=0 everywhere, el32=1) ----
    lgx = logits.copy()
    lgx[~tmask] = -np.inf
    el_full = np.exp(lgx - CSHIFT)                                   # [B,T,L]
    el_full = np.concatenate([el_full, np.zeros((B, 1, L), np.float32)], axis=1)
    el32 = (np.arange(T + 1)[None, :] >= lens[:, None]).astype(np.float32)  # [B,T+1]

    bf = ml_dtypes.bfloat16
    gsl = [(0, 43), (43, 86), (86, 128)]  # local seq ranges per label group
    el_cores, u0_cores = [], []
    for core in range(NCORES):
        b0 = core * BPC
        E = el_full[b0 : b0 + BPC]          # [128, T+1, L]
        E32 = el32[b0 : b0 + BPC]           # [128, T+1]
        packed = np.zeros((NPART, SP, COLS), np.float32)
        u0 = np.zeros((NPART, NCOL), np.float32)
        for c in range(C):
            t0 = c * S
            sl = E[:, t0 + 1 : t0 + SP + 1, :]    # [128, SP, L]
            sl32 = E32[:, t0 + 1 : t0 + SP + 1]   # [128, SP]
            for g, (s0, s1) in enumerate(gsl):
                nc_ = s1 - s0
                cc = NCOL * c
                packed[32 * g : 32 * g + 32, :, cc : cc + nc_] = sl[s0:s1].transpose(2, 1, 0)
                packed[NACT + g, :, cc : cc + nc_] = sl32[s0:s1].T
                if c == 0:
                    u0[32 * g : 32 * g + 32, :nc_] = E[s0:s1, 0, :].T
        el_cores.append(packed.astype(bf))
        u0_cores.append(u0.astype(bf))

    # ---- stationary operator: block-diag exp(trans) + sink + colsum ----
    Ew = np.exp(trans).astype(np.float32)
    Wf = np.zeros((NPART, MOUT), np.float32)
    for g in range(3):
        a, sk, cs = 32 * g, NACT + g, NPART + g
        Wf[a : a + 32, a : a + 32] = Ew
        Wf[a : a + 32, sk] = 1.0
        Wf[sk, sk] = 1.0
        Wf[a : a + 32, cs] = 1.0
        Wf[sk, cs] = 1.0
    return gp, lens, el_cores, u0_cores, Wf.astype(bf)


def _log(msg):
    import time as _t

    print(f"[kernel {_t.strftime('%H:%M:%S')}] {msg}", flush=True)


def kernel(logits, trans, labels, seq_lens):
    global last_result
    from concourse.bass_utils import run_bass_kernel_spmd

    _log("host prep start")
    gp, lens, el_cores, u0_cores, Wf = _host_prep(logits, trans, labels, seq_lens)
    _log("host prep done")

    if "nc" not in _prog_cache:
        _prog_cache["nc"] = _build_program()
        _log("program built")
    nc = _prog_cache["nc"]

    in_maps = [
        {"el": el_cores[i], "u0": u0_cores[i], "wf": Wf}
        for i in range(NCORES)
    ]
    r = run_bass_kernel_spmd(nc, in_maps, core_ids=list(range(NCORES)))
    last_result = r
    _log("device run done")

    # ---- unshard: per-core [3,COLS]/[6,COLS] -> per-sequence chain arrays ----
    gsl = [(0, 43), (43, 86), (86, 128)]
    colE = np.zeros((C, B), np.float64)   # chain colsum at its tick BURN
    colF = np.zeros((C, B), np.float64)   # chain colsum at its final tick
    sinkF = np.zeros((C, B), np.float64)  # chain sink at its final tick
    for core in range(NCORES):
        sE = np.asarray(last_result.results[core]["snapE"], np.float64)  # [6,COLS]
        sF = np.asarray(last_result.results[core]["snapF"], np.float64)  # [6,COLS]
        b0 = core * BPC
        for g, (s0, s1) in enumerate(gsl):
            nc_ = s1 - s0
            colE[:, b0 + s0 : b0 + s1] = sE[3 + g].reshape(C, NCOL)[:, :nc_]
            sinkF[:, b0 + s0 : b0 + s1] = sF[g].reshape(C, NCOL)[:, :nc_]
            colF[:, b0 + s0 : b0 + s1] = sF[3 + g].reshape(C, NCOL)[:, :nc_]

    # ---- stitch scales: chain c valid for len in (c*S+BURN, c*S+SP] ----
    j = np.zeros(B, np.int64)
    for c in range(1, C):
        j[lens > c * S + BURN] = c
    with np.errstate(divide="ignore", invalid="ignore"):
        log_rho = np.log(colE[1:]) - np.log(colF[:-1])        # [C-1, B]
        log_gamma = np.concatenate(
            [np.zeros((1, B)), np.cumsum(log_rho, axis=0)], axis=0
        )                                                      # [C, B]
        log_sink = np.log(sinkF[j, np.arange(B)])
    logZ = log_sink - log_gamma[j, np.arange(B)] + CSHIFT * lens
    return (gp - logZ).astype(np.float32)


# revision 9
# speedup vs baseline: 1.3157x; 1.3157x over previous
"""CRF log-likelihood kernel for Trainium2 (Bass/Tile), 8-core data parallel.

out[b] = gold_path_score(b) - logZ(b)

logZ via chunked cold-start forward chains (exp domain).  The transfer
operator D_{el_t} Wf^T is strictly positive, so it contracts directions at
~0.3/step (Birkhoff): a chain started from the uniform vector forgets its
start after ~10 steps.  Split t = 1..T into C chunks of S steps; chain c
starts cold at t = c*S and runs BURN warm-up ticks + S valid ticks, all C
chains advancing in lockstep columns of the same matmul.  Serial depth drops
from T to SP = BURN + S ticks.

Scale recovery: chain c's state is gamma_c * alpha_t (unknown scalar
gamma_c).  At boundary time c*S + SP - 1 both chain c (final tick) and chain
c+1 (tick BURN) hold the same alpha direction, so the ratio of their Wf
column sums gives gamma_{c+1}/gamma_c exactly.  Sink rows capture
sum(alpha_{len-1}) at t == len per sequence (el32 gating), persist to the
chain end, and are read from the final tick's PSUM.  Host stitches:
logZ = log(sink_j) - log(gamma_j) + CSHIFT*len,  j = chunk containing len.

Per-core layout (128 sequences): partitions 0..95 = 3 label groups x 32,
96..98 = per-group sink rows; psum rows 99..101 = per-group column sums.
Columns: chain c occupies cols [43c, 43c+43); within a column, the 3 label
groups hold 3 different sequences (43+43+42+pad = 128).  Two column groups
of 10 chains each give two independent MM->TT streams that overlap on
PE/DVE.  Emissions exp'd on host, shipped bf16 (4x less DMA than f32
logits; el rounding is ~0.4% -> ~0.1 absolute logZ noise over 512 steps).
"""

import numpy as np
import ml_dtypes

B, T, L = 1024, 512, 32
NCORES = 8
BPC = B // NCORES        # 128 sequences per core
NCOL = 43                # columns per chain (3 label groups: 43+43+42+pad)
NACT = 96                # active label partitions
NPART = 99               # + 3 sink rows
MOUT = 102               # + 3 colsum rows
CSHIFT = 4.5
C = 22                   # chains (chunks)
BURN = 6                 # cold-start warm-up ticks
S = (T - BURN) // C      # valid ticks per chain (23)
assert BURN + C * S == T
SP = BURN + S            # ticks per chain (29)
COLS = NCOL * C          # 946 total columns
GSPLIT = 11 * NCOL       # group A/B: 11 chains (473 cols) each

_prog_cache = {}
last_result = None       # BassKernelResults of the most recent run (for test.py)


def _build_program():
    import concourse.bacc as bacc
    import concourse.tile as tile
    from concourse import mybir

    f32 = mybir.dt.float32
    bf16 = mybir.dt.bfloat16
    AF = mybir.ActivationFunctionType

    nc = bacc.Bacc("TRN2", target_bir_lowering=False, debug=False, num_devices=NCORES)
    el_d = nc.dram_tensor("el", [NPART, SP, COLS], bf16, kind="ExternalInput")
    u0_d = nc.dram_tensor("u0", [NPART, NCOL], bf16, kind="ExternalInput")
    wf_d = nc.dram_tensor("wf", [NPART, MOUT], bf16, kind="ExternalInput")
    snapE_d = nc.dram_tensor("snapE", [6, COLS], f32, kind="ExternalOutput")
    snapF_d = nc.dram_tensor("snapF", [6, COLS], f32, kind="ExternalOutput")

    with tile.TileContext(nc) as tc:
        with (
            tc.tile_pool(name="consts", bufs=1) as consts,
            tc.tile_pool(name="elp", bufs=1) as elp,
            tc.tile_pool(name="uA", bufs=3) as uApool,
            tc.tile_pool(name="uB", bufs=3) as uBpool,
            tc.tile_pool(name="fin", bufs=1) as fin,
            tc.tile_pool(name="psA", bufs=3, space="PSUM") as psApool,
            tc.tile_pool(name="psB", bufs=3, space="PSUM") as psBpool,
        ):
            wf_sb = consts.tile([NPART, MOUT], bf16)
            u0_sb = consts.tile([NPART, COLS], bf16)
            el_sb = elp.tile([NPART, SP, COLS], bf16)
            finE = fin.tile([6, COLS], f32)
            finF = fin.tile([6, COLS], f32)

            # cold-start state is uniform 1 (sinks 0) everywhere except the
            # chain-0 block, which is DMA'd (= alpha_0); memsets keep the big
            # u0 off the DMA critical path.
            nc.vector.memset(u0_sb[:], 1.0)
            nc.vector.memset(u0_sb[NACT:NPART, :], 0.0)
            # wf + u0 block gate the first MM: sync queue, first.
            nc.sync.dma_start(out=wf_sb[:], in_=wf_d[:])
            nc.sync.dma_start(out=u0_sb[:, 0:NCOL], in_=u0_d[:])
            # el streams in tick-major chunks on the sync queue (the one
            # with the full DMA engine fan-out); first chunks small so the
            # chain starts early.
            edges = [0, 1, 2, 4, 6, 9, 12, 16, 20, 24, SP]
            for k0, k1 in zip(edges[:-1], edges[1:]):
                nc.sync.dma_start(
                    out=el_sb[:, k0:k1, :], in_=el_d[:, k0:k1, :]
                )

            groups = [
                (uApool, psApool, 0, GSPLIT),
                (uBpool, psBpool, GSPLIT, COLS),
            ]
            uprev = [u0_sb[:, c0:c1] for (_, _, c0, c1) in groups]
            for k in range(1, SP + 1):
                for gi, (upool, pspool, c0, c1) in enumerate(groups):
                    ps = pspool.tile([MOUT, c1 - c0], f32, tag=f"ps{gi}")
                    nc.tensor.matmul(ps[:], wf_sb[:], uprev[gi], start=True, stop=True)
                    if k < SP:
                        un = upool.tile([NPART, c1 - c0], bf16, tag=f"u{gi}")
                        nc.vector.tensor_mul(
                            un[:], ps[0:NPART, :], el_sb[:, k - 1, c0:c1]
                        )
                        uprev[gi] = un[:]
                    if k == BURN:
                        nc.scalar.activation(
                            finE[:, c0:c1], ps[NACT:MOUT, :], AF.Copy
                        )
                    if k == SP:
                        nc.scalar.activation(
                            finF[:, c0:c1], ps[NACT:MOUT, :], AF.Copy
                        )
                        nc.sync.dma_start(
                            out=snapF_d[:, c0:c1], in_=finF[:, c0:c1]
                        )
            nc.sync.dma_start(out=snapE_d[:], in_=finE[:])

    nc.compile()
    return nc


def _host_prep(logits, trans, labels, seq_lens):
    logits = np.ascontiguousarray(np.asarray(logits), dtype=np.float32)
    trans = np.asarray(trans, dtype=np.float32)
    labels = np.asarray(labels)
    lens = np.clip(np.asarray(seq_lens), 1, T).astype(np.int64)

    # ---- gold path score (host: index gathers over small inputs) ----
    tmask = np.arange(T)[None, :] < lens[:, None]
    unary = np.take_along_axis(logits, labels[..., None].astype(np.int64), axis=2)[..., 0]
    gp = (unary * tmask).sum(1) + (trans[labels[:, :-1], labels[:, 1:]] * tmask[:, 1:]).sum(1)

    # ---- emissions: exp on host, masked past seq end; slice t=T is
    # capture-only (el=0 everywhere, el32=1) ----
    lgx = logits.copy()
    lgx[~tmask] = -np.inf
    el_full = np.exp(lgx - CSHIFT)                                   # [B,T,L]
    el_full = np.concatenate([el_full, np.zeros((B, 1, L), np.float32)], axis=1)
    el32 = (np.arange(T + 1)[None, :] >= lens[:, None]).astype(np.float32)  # [B,T+1]

    bf = ml_dtypes.bfloat16
    gsl = [(0, 43), (43, 86), (86, 128)]  # local seq ranges per label group
    el_cores, u0_cores = [], []
    for core in range(NCORES):
        b0 = core * BPC
        E = el_full[b0 : b0 + BPC]          # [128, T+1, L]
        E32 = el32[b0 : b0 + BPC]           # [128, T+1]
        packed = np.zeros((NPART, SP, COLS), np.float32)
        u0 = np.zeros((NPART, NCOL), np.float32)
        for c in range(C):
            t0 = c * S
            sl = E[:, t0 + 1 : t0 + SP + 1, :]    # [128, SP, L]
            sl32 = E32[:, t0 + 1 : t0 + SP + 1]   # [128, SP]
            for g, (s0, s1) in enumerate(gsl):
                nc_ = s1 - s0
                cc = NCOL * c
                packed[32 * g : 32 * g + 32, :, cc : cc + nc_] = sl[s0:s1].transpose(2, 1, 0)
                packed[NACT + g, :, cc : cc + nc_] = sl32[s0:s1].T
                if c == 0:
                    u0[32 * g : 32 * g + 32, :nc_] = E[s0:s1, 0, :].T
        el_cores.append(packed.astype(bf))
        u0_cores.append(u0.astype(bf))

    # ---- stationary operator: block-diag exp(trans) + sink + colsum ----
    Ew = np.exp(trans).astype(np.float32)
    Wf = np.zeros((NPART, MOUT), np.float32)
    for g in range(3):
        a, sk, cs = 32 * g, NACT + g, NPART + g
        Wf[a : a + 32, a : a + 32] = Ew
        Wf[a : a + 32, sk] = 1.0
        Wf[sk, sk] = 1.0
        Wf[a : a + 32, cs] = 1.0
        Wf[sk, cs] = 1.0
    return gp, lens, el_cores, u0_cores, Wf.astype(bf)


def _log(msg):
    import time as _t

    print(f"[kernel {_t.strftime('%H:%M:%S')}] {msg}", flush=True)


def kernel(logits, trans, labels, seq_lens):
    global last_result
    from concourse.bass_utils import run_bass_kernel_spmd

    _log("host prep start")
    gp, lens, el_cores, u0_cores, Wf = _host_prep(logits, trans, labels, seq_lens)
    _log("host prep done")

    if "nc" not in _prog_cache:
        _prog_cache["nc"] = _build_program()
        _log("program built")
    nc = _prog_cache["nc"]

    in_maps = [
        {"el": el_cores[i], "u0": u0_cores[i], "wf": Wf}
        for i in range(NCORES)
    ]
    r = run_bass_kernel_spmd(nc, in_maps, core_ids=list(range(NCORES)))
    last_result = r
    _log("device run done")

    # ---- unshard: per-core [3,COLS]/[6,COLS] -> per-sequence chain arrays ----
    gsl = [(0, 43), (43, 86), (86, 128)]
    colE = np.zeros((C, B), np.float64)   # chain colsum at its tick BURN
    colF = np.zeros((C, B), np.float64)   # chain colsum at its final tick
    sinkF = np.zeros((C, B), np.float64)  # chain sink at its final tick
    for core in range(NCORES):
        sE = np.asarray(last_result.results[core]["snapE"], np.float64)  # [6,COLS]
        sF = np.asarray(last_result.results[core]["snapF"], np.float64)  # [6,COLS]
        b0 = core * BPC
        for g, (s0, s1) in enumerate(gsl):
            nc_ = s1 - s0
            colE[:, b0 + s0 : b0 + s1] = sE[3 + g].reshape(C, NCOL)[:, :nc_]
            sinkF[:, b0 + s0 : b0 + s1] = sF[g].reshape(C, NCOL)[:, :nc_]
            colF[:, b0 + s0 : b0 + s1] = sF[3 + g].reshape(C, NCOL)[:, :nc_]

    # ---- stitch scales: chain c valid for len in (c*S+BURN, c*S+SP] ----
    j = np.zeros(B, np.int64)
    for c in range(1, C):
        j[lens > c * S + BURN] = c
    with np.errstate(divide="ignore", invalid="ignore"):
        log_rho = np.log(colE[1:]) - np.log(colF[:-1])        # [C-1, B]
        log_gamma = np.concatenate(
            [np.zeros((1, B)), np.cumsum(log_rho, axis=0)], axis=0
        )                                                      # [C, B]
        log_sink = np.log(sinkF[j, np.arange(B)])
    logZ = log_sink - log_gamma[j, np.arange(B)] + CSHIFT * lens
    return (gp - logZ).astype(np.float32)


# revision 10
# speedup vs baseline: 1.3362x; 1.0156x over previous
"""CRF log-likelihood kernel for Trainium2 (Bass/Tile), 8-core data parallel.

out[b] = gold_path_score(b) - logZ(b)

logZ via chunked cold-start forward chains (exp domain).  The transfer
operator D_{el_t} Wf^T is strictly positive, so it contracts directions at
~0.3/step (Birkhoff): a chain started from the uniform vector forgets its
start after ~10 steps.  Split t = 1..T into C chunks of S steps; chain c
starts cold at t = c*S and runs BURN warm-up ticks + S valid ticks, all C
chains advancing in lockstep columns of the same matmul.  Serial depth drops
from T to SP = BURN + S ticks.

Scale recovery: chain c's state is gamma_c * alpha_t (unknown scalar
gamma_c).  At boundary time c*S + SP - 1 both chain c (final tick) and chain
c+1 (tick BURN) hold the same alpha direction, so the ratio of their Wf
column sums gives gamma_{c+1}/gamma_c exactly.  Sink rows capture
sum(alpha_{len-1}) at t == len per sequence (el32 gating), persist to the
chain end, and are read from the final tick's PSUM.  Host stitches:
logZ = log(sink_j) - log(gamma_j) + CSHIFT*len,  j = chunk containing len.

Per-core layout (128 sequences): partitions 0..95 = 3 label groups x 32,
96..98 = per-group sink rows; psum rows 99..101 = per-group column sums.
Columns: chain c occupies cols [43c, 43c+43); within a column, the 3 label
groups hold 3 different sequences (43+43+42+pad = 128).  Two column groups
of 10 chains each give two independent MM->TT streams that overlap on
PE/DVE.  Emissions exp'd on host, shipped bf16 (4x less DMA than f32
logits; el rounding is ~0.4% -> ~0.1 absolute logZ noise over 512 steps).
"""

import numpy as np
import ml_dtypes

B, T, L = 1024, 512, 32
NCORES = 8
BPC = B // NCORES        # 128 sequences per core
NCOL = 43                # columns per chain (3 label groups: 43+43+42+pad)
NACT = 96                # active label partitions
NPART = 99               # + 3 sink rows
MOUT = 102               # + 3 colsum rows
CSHIFT = 4.5
C = 22                   # chains (chunks)
BURN = 6                 # cold-start warm-up ticks
S = (T - BURN) // C      # valid ticks per chain (23)
assert BURN + C * S == T
SP = BURN + S            # ticks per chain (29)
COLS = NCOL * C          # 946 total columns
GSPLIT = 11 * NCOL       # group A/B: 11 chains (473 cols) each

_prog_cache = {}
last_result = None       # BassKernelResults of the most recent run (for test.py)


def _build_program():
    import concourse.bacc as bacc
    import concourse.tile as tile
    from concourse import mybir

    f32 = mybir.dt.float32
    bf16 = mybir.dt.bfloat16
    AF = mybir.ActivationFunctionType

    nc = bacc.Bacc("TRN2", target_bir_lowering=False, debug=False, num_devices=NCORES)
    el_d = nc.dram_tensor("el", [NPART, SP, COLS], bf16, kind="ExternalInput")
    u0_d = nc.dram_tensor("u0", [NPART, NCOL], bf16, kind="ExternalInput")
    wf_d = nc.dram_tensor("wf", [NPART, MOUT], bf16, kind="ExternalInput")
    snapE_d = nc.dram_tensor("snapE", [6, COLS], f32, kind="ExternalOutput")
    snapF_d = nc.dram_tensor("snapF", [6, COLS], f32, kind="ExternalOutput")

    with tile.TileContext(nc) as tc:
        with (
            tc.tile_pool(name="consts", bufs=1) as consts,
            tc.tile_pool(name="elp", bufs=1) as elp,
            tc.tile_pool(name="uA", bufs=3) as uApool,
            tc.tile_pool(name="uB", bufs=3) as uBpool,
            tc.tile_pool(name="fin", bufs=1) as fin,
            tc.tile_pool(name="psA", bufs=3, space="PSUM") as psApool,
            tc.tile_pool(name="psB", bufs=3, space="PSUM") as psBpool,
        ):
            wf_sb = consts.tile([NPART, MOUT], bf16)
            u0_sb = consts.tile([NPART, COLS], bf16)
            el_sb = elp.tile([NPART, SP, COLS], bf16)
            finE = fin.tile([6, COLS], f32)
            finF = fin.tile([6, COLS], f32)

            # cold-start state is uniform 1 (sinks 0) everywhere except the
            # chain-0 block, which is DMA'd (= alpha_0); memsets keep the big
            # u0 off the DMA critical path.
            nc.vector.memset(u0_sb[:], 1.0)
            nc.vector.memset(u0_sb[NACT:NPART, :], 0.0)
            # wf + u0 block gate the first MM: sync queue, first.
            nc.sync.dma_start(out=wf_sb[:], in_=wf_d[:])
            nc.sync.dma_start(out=u0_sb[:, 0:NCOL], in_=u0_d[:])
            # el streams in tick-major chunks on the sync queue (the one
            # with the full DMA engine fan-out); first chunks small so the
            # chain starts early.
            edges = [0, 1, 2, 3, 4, 5, 6, 8, 10, 12, 15, 18, 22, 26, SP]
            for k0, k1 in zip(edges[:-1], edges[1:]):
                nc.sync.dma_start(
                    out=el_sb[:, k0:k1, :], in_=el_d[:, k0:k1, :]
                )

            groups = [
                (uApool, psApool, 0, GSPLIT),
                (uBpool, psBpool, GSPLIT, COLS),
            ]
            uprev = [u0_sb[:, c0:c1] for (_, _, c0, c1) in groups]
            for k in range(1, SP + 1):
                for gi, (upool, pspool, c0, c1) in enumerate(groups):
                    ps = pspool.tile([MOUT, c1 - c0], f32, tag=f"ps{gi}")
                    nc.tensor.matmul(ps[:], wf_sb[:], uprev[gi], start=True, stop=True)
                    if k < SP:
                        un = upool.tile([NPART, c1 - c0], bf16, tag=f"u{gi}")
                        nc.vector.tensor_mul(
                            un[:], ps[0:NPART, :], el_sb[:, k - 1, c0:c1]
                        )
                        uprev[gi] = un[:]
                    if k == BURN:
                        nc.scalar.activation(
                            finE[:, c0:c1], ps[NACT:MOUT, :], AF.Copy
                        )
                    if k == SP:
                        nc.scalar.activation(
                            finF[:, c0:c1], ps[NACT:MOUT, :], AF.Copy
                        )
                        nc.sync.dma_start(
                            out=snapF_d[:, c0:c1], in_=finF[:, c0:c1]
                        )
            nc.sync.dma_start(out=snapE_d[:], in_=finE[:])

    nc.compile()
    return nc


def _host_prep(logits, trans, labels, seq_lens):
    logits = np.ascontiguousarray(np.asarray(logits), dtype=np.float32)
    trans = np.asarray(trans, dtype=np.float32)
    labels = np.asarray(labels)
    lens = np.clip(np.asarray(seq_lens), 1, T).astype(np.int64)

    # ---- gold path score (host: index gathers over small inputs) ----
    tmask = np.arange(T)[None, :] < lens[:, None]
    unary = np.take_along_axis(logits, labels[..., None].astype(np.int64), axis=2)[..., 0]
    gp = (unary * tmask).sum(1) + (trans[labels[:, :-1], labels[:, 1:]] * tmask[:, 1:]).sum(1)

    # ---- emissions: exp on host, masked past seq end; slice t=T is
    # capture-only (el=0 everywhere, el32=1) ----
    lgx = logits.copy()
    lgx[~tmask] = -np.inf
    el_full = np.exp(lgx - CSHIFT)                                   # [B,T,L]
    el_full = np.concatenate([el_full, np.zeros((B, 1, L), np.float32)], axis=1)
    el32 = (np.arange(T + 1)[None, :] >= lens[:, None]).astype(np.float32)  # [B,T+1]

    bf = ml_dtypes.bfloat16
    gsl = [(0, 43), (43, 86), (86, 128)]  # local seq ranges per label group
    el_cores, u0_cores = [], []
    for core in range(NCORES):
        b0 = core * BPC
        E = el_full[b0 : b0 + BPC]          # [128, T+1, L]
        E32 = el32[b0 : b0 + BPC]           # [128, T+1]
        packed = np.zeros((NPART, SP, COLS), np.float32)
        u0 = np.zeros((NPART, NCOL), np.float32)
        for c in range(C):
            t0 = c * S
            sl = E[:, t0 + 1 : t0 + SP + 1, :]    # [128, SP, L]
            sl32 = E32[:, t0 + 1 : t0 + SP + 1]   # [128, SP]
            for g, (s0, s1) in enumerate(gsl):
                nc_ = s1 - s0
                cc = NCOL * c
                packed[32 * g : 32 * g + 32, :, cc : cc + nc_] = sl[s0:s1].transpose(2, 1, 0)
                packed[NACT + g, :, cc : cc + nc_] = sl32[s0:s1].T
                if c == 0:
                    u0[32 * g : 32 * g + 32, :nc_] = E[s0:s1, 0, :].T
        el_cores.append(packed.astype(bf))
        u0_cores.append(u0.astype(bf))

    # ---- stationary operator: block-diag exp(trans) + sink + colsum ----
    Ew = np.exp(trans).astype(np.float32)
    Wf = np.zeros((NPART, MOUT), np.float32)
    for g in range(3):
        a, sk, cs = 32 * g, NACT + g, NPART + g
        Wf[a : a + 32, a : a + 32] = Ew
        Wf[a : a + 32, sk] = 1.0
        Wf[sk, sk] = 1.0
        Wf[a : a + 32, cs] = 1.0
        Wf[sk, cs] = 1.0
    return gp, lens, el_cores, u0_cores, Wf.astype(bf)


def _log(msg):
    import time as _t

    print(f"[kernel {_t.strftime('%H:%M:%S')}] {msg}", flush=True)


def kernel(logits, trans, labels, seq_lens):
    global last_result
    from concourse.bass_utils import run_bass_kernel_spmd

    _log("host prep start")
    gp, lens, el_cores, u0_cores, Wf = _host_prep(logits, trans, labels, seq_lens)
    _log("host prep done")

    if "nc" not in _prog_cache:
        _prog_cache["nc"] = _build_program()
        _log("program built")
    nc = _prog_cache["nc"]

    in_maps = [
        {"el": el_cores[i], "u0": u0_cores[i], "wf": Wf}
        for i in range(NCORES)
    ]
    r = run_bass_kernel_spmd(nc, in_maps, core_ids=list(range(NCORES)))
    last_result = r
    _log("device run done")

    # ---- unshard: per-core [3,COLS]/[6,COLS] -> per-sequence chain arrays ----
    gsl = [(0, 43), (43, 86), (86, 128)]
    colE = np.zeros((C, B), np.float64)   # chain colsum at its tick BURN
    colF = np.zeros((C, B), np.float64)   # chain colsum at its final tick
    sinkF = np.zeros((C, B), np.float64)  # chain sink at its final tick
    for core in range(NCORES):
        sE = np.asarray(last_result.results[core]["snapE"], np.float64)  # [6,COLS]
        sF = np.asarray(last_result.results[core]["snapF"], np.float64)  # [6,COLS]
        b0 = core * BPC
        for g, (s0, s1) in enumerate(gsl):
            nc_ = s1 - s0
            colE[:, b0 + s0 : b0 + s1] = sE[3 + g].reshape(C, NCOL)[:, :nc_]
            sinkF[:, b0 + s0 : b0 + s1] = sF[g].reshape(C, NCOL)[:, :nc_]
            colF[:, b0 + s0 : b0 + s1] = sF[3 + g].reshape(C, NCOL)[:, :nc_]

    # ---- stitch scales: chain c valid for len in (c*S+BURN, c*S+SP] ----
    j = np.zeros(B, np.int64)
    for c in range(1, C):
        j[lens > c * S + BURN] = c
    with np.errstate(divide="ignore", invalid="ignore"):
        log_rho = np.log(colE[1:]) - np.log(colF[:-1])        # [C-1, B]
        log_gamma = np.concatenate(
            [np.zeros((1, B)), np.cumsum(log_rho, axis=0)], axis=0
        )                                                      # [C, B]
        log_sink = np.log(sinkF[j, np.arange(B)])
    logZ = log_sink - log_gamma[j, np.arange(B)] + CSHIFT * lens
    return (gp - logZ).astype(np.float32)


# revision 11
# speedup vs baseline: 1.3511x; 1.0111x over previous
"""CRF log-likelihood kernel for Trainium2 (Bass/Tile), 8-core data parallel.

out[b] = gold_path_score(b) - logZ(b)

logZ via chunked cold-start forward chains (exp domain).  The transfer
operator D_{el_t} Wf^T is strictly positive, so it contracts directions at
~0.3/step (Birkhoff): a chain started from the uniform vector forgets its
start after ~10 steps.  Split t = 1..T into C chunks of S steps; chain c
starts cold at t = c*S and runs BURN warm-up ticks + S valid ticks, all C
chains advancing in lockstep columns of the same matmul.  Serial depth drops
from T to SP = BURN + S ticks.

Scale recovery: chain c's state is gamma_c * alpha_t (unknown scalar
gamma_c).  At boundary time c*S + SP - 1 both chain c (final tick) and chain
c+1 (tick BURN) hold the same alpha direction, so the ratio of their Wf
column sums gives gamma_{c+1}/gamma_c exactly.  Sink rows capture
sum(alpha_{len-1}) at t == len per sequence (el32 gating), persist to the
chain end, and are read from the final tick's PSUM.  Host stitches:
logZ = log(sink_j) - log(gamma_j) + CSHIFT*len,  j = chunk containing len.

Per-core layout (128 sequences): partitions 0..95 = 3 label groups x 32,
96..98 = per-group sink rows; psum rows 99..101 = per-group column sums.
Columns: chain c occupies cols [43c, 43c+43); within a column, the 3 label
groups hold 3 different sequences (43+43+42+pad = 128).  Two column groups
of 10 chains each give two independent MM->TT streams that overlap on
PE/DVE.  Emissions exp'd on host, shipped bf16 (4x less DMA than f32
logits; el rounding is ~0.4% -> ~0.1 absolute logZ noise over 512 steps).
"""

import numpy as np
import ml_dtypes

B, T, L = 1024, 512, 32
NCORES = 8
BPC = B // NCORES        # 128 sequences per core
NCOL = 43                # columns per chain (3 label groups: 43+43+42+pad)
NACT = 96                # active label partitions
NPART = 99               # + 3 sink rows
MOUT = 102               # + 3 colsum rows
CSHIFT = 4.5
C = 22                   # chains (chunks)
BURN = 6                 # cold-start warm-up ticks
S = (T - BURN) // C      # valid ticks per chain (23)
assert BURN + C * S == T
SP = BURN + S            # ticks per chain (29)
COLS = NCOL * C          # 946 total columns
GSPLIT = 11 * NCOL       # group A/B: 11 chains (473 cols) each

_prog_cache = {}
last_result = None       # BassKernelResults of the most recent run (for test.py)


def _build_program():
    import concourse.bacc as bacc
    import concourse.tile as tile
    from concourse import mybir

    f32 = mybir.dt.float32
    bf16 = mybir.dt.bfloat16
    AF = mybir.ActivationFunctionType

    nc = bacc.Bacc("TRN2", target_bir_lowering=False, debug=False, num_devices=NCORES)
    el_d = nc.dram_tensor("el", [NPART, SP - 1, COLS], bf16, kind="ExternalInput")
    u0_d = nc.dram_tensor("u0", [NPART, NCOL], bf16, kind="ExternalInput")
    wf_d = nc.dram_tensor("wf", [NPART, MOUT], bf16, kind="ExternalInput")
    uE_d = nc.dram_tensor("uE", [NPART, COLS], bf16, kind="ExternalOutput")
    uF_d = nc.dram_tensor("uF", [NPART, COLS], bf16, kind="ExternalOutput")

    with tile.TileContext(nc) as tc:
        with (
            tc.tile_pool(name="consts", bufs=1) as consts,
            tc.tile_pool(name="elp", bufs=1) as elp,
            tc.tile_pool(name="uA", bufs=3) as uApool,
            tc.tile_pool(name="uB", bufs=3) as uBpool,
            tc.tile_pool(name="fin", bufs=1) as fin,
            tc.tile_pool(name="psA", bufs=3, space="PSUM") as psApool,
            tc.tile_pool(name="psB", bufs=3, space="PSUM") as psBpool,
        ):
            wf_sb = consts.tile([NPART, MOUT], bf16)
            u0_sb = consts.tile([NPART, COLS], bf16)
            el_sb = elp.tile([NPART, SP - 1, COLS], bf16)
            uE_sb = fin.tile([NPART, COLS], bf16)

            # cold-start state is uniform 1 (sinks 0) everywhere except the
            # chain-0 block, which is DMA'd (= alpha_0); memsets keep the big
            # u0 off the DMA critical path.
            nc.vector.memset(u0_sb[:], 1.0)
            nc.vector.memset(u0_sb[NACT:NPART, :], 0.0)
            # wf + u0 block gate the first MM: sync queue, first.
            nc.sync.dma_start(out=wf_sb[:], in_=wf_d[:])
            nc.sync.dma_start(out=u0_sb[:, 0:NCOL], in_=u0_d[:])
            # el streams in tick-major chunks on the sync queue (the one
            # with the full DMA engine fan-out); first chunks small so the
            # chain starts early.
            edges = [0, 1, 2, 3, 4, 5, 6, 8, 10, 12, 15, 18, 22, 26, SP - 1]
            for k0, k1 in zip(edges[:-1], edges[1:]):
                nc.sync.dma_start(
                    out=el_sb[:, k0:k1, :], in_=el_d[:, k0:k1, :]
                )

            groups = [
                (uApool, psApool, 0, GSPLIT),
                (uBpool, psBpool, GSPLIT, COLS),
            ]
            # ticks 1..SP-1; the boundary column sums (for scale stitching)
            # are recovered on the host from the raw states at tick BURN-1
            # (saved via an off-path ACT copy) and tick SP-1 (the final
            # state): captured-Z == column sum at the capture tick, so the
            # state tiles carry everything.
            uprev = [u0_sb[:, c0:c1] for (_, _, c0, c1) in groups]
            for k in range(1, SP):
                for gi, (upool, pspool, c0, c1) in enumerate(groups):
                    ps = pspool.tile([MOUT, c1 - c0], f32, tag=f"ps{gi}")
                    nc.tensor.matmul(ps[:], wf_sb[:], uprev[gi], start=True, stop=True)
                    un = upool.tile([NPART, c1 - c0], bf16, tag=f"u{gi}")
                    nc.vector.tensor_mul(
                        un[:], ps[0:NPART, :], el_sb[:, k - 1, c0:c1]
                    )
                    uprev[gi] = un[:]
                    if k == BURN - 1:
                        nc.scalar.activation(
                            uE_sb[:, c0:c1], un[:], AF.Copy
                        )
            for gi, (_, _, c0, c1) in enumerate(groups):
                nc.sync.dma_start(out=uF_d[:, c0:c1], in_=uprev[gi])
            nc.sync.dma_start(out=uE_d[:], in_=uE_sb[:])

    nc.compile()
    return nc


def _host_prep(logits, trans, labels, seq_lens):
    logits = np.ascontiguousarray(np.asarray(logits), dtype=np.float32)
    trans = np.asarray(trans, dtype=np.float32)
    labels = np.asarray(labels)
    lens = np.clip(np.asarray(seq_lens), 1, T).astype(np.int64)

    # ---- gold path score (host: index gathers over small inputs) ----
    tmask = np.arange(T)[None, :] < lens[:, None]
    unary = np.take_along_axis(logits, labels[..., None].astype(np.int64), axis=2)[..., 0]
    gp = (unary * tmask).sum(1) + (trans[labels[:, :-1], labels[:, 1:]] * tmask[:, 1:]).sum(1)

    # ---- emissions: exp on host, masked past seq end; slice t=T is
    # capture-only (el=0 everywhere, el32=1) ----
    lgx = logits.copy()
    lgx[~tmask] = -np.inf
    el_full = np.exp(lgx - CSHIFT)                                   # [B,T,L]
    el_full = np.concatenate([el_full, np.zeros((B, 1, L), np.float32)], axis=1)
    el32 = (np.arange(T + 1)[None, :] >= lens[:, None]).astype(np.float32)  # [B,T+1]

    bf = ml_dtypes.bfloat16
    gsl = [(0, 43), (43, 86), (86, 128)]  # local seq ranges per label group
    el_cores, u0_cores = [], []
    for core in range(NCORES):
        b0 = core * BPC
        E = el_full[b0 : b0 + BPC]          # [128, T+1, L]
        E32 = el32[b0 : b0 + BPC]           # [128, T+1]
        packed = np.zeros((NPART, SP - 1, COLS), np.float32)
        u0 = np.zeros((NPART, NCOL), np.float32)
        for c in range(C):
            t0 = c * S
            sl = E[:, t0 + 1 : t0 + SP, :]     # [128, SP-1, L]
            sl32 = E32[:, t0 + 1 : t0 + SP]    # [128, SP-1]
            for g, (s0, s1) in enumerate(gsl):
                nc_ = s1 - s0
                cc = NCOL * c
                packed[32 * g : 32 * g + 32, :, cc : cc + nc_] = sl[s0:s1].transpose(2, 1, 0)
                packed[NACT + g, :, cc : cc + nc_] = sl32[s0:s1].T
                if c == 0:
                    u0[32 * g : 32 * g + 32, :nc_] = E[s0:s1, 0, :].T
        el_cores.append(packed.astype(bf))
        u0_cores.append(u0.astype(bf))

    # ---- stationary operator: block-diag exp(trans) + sink + colsum ----
    Ew = np.exp(trans).astype(np.float32)
    Wf = np.zeros((NPART, MOUT), np.float32)
    for g in range(3):
        a, sk, cs = 32 * g, NACT + g, NPART + g
        Wf[a : a + 32, a : a + 32] = Ew
        Wf[a : a + 32, sk] = 1.0
        Wf[sk, sk] = 1.0
        Wf[a : a + 32, cs] = 1.0
        Wf[sk, cs] = 1.0
    return gp, lens, el_cores, u0_cores, Wf.astype(bf)


def _log(msg):
    import time as _t

    print(f"[kernel {_t.strftime('%H:%M:%S')}] {msg}", flush=True)


def kernel(logits, trans, labels, seq_lens):
    global last_result
    from concourse.bass_utils import run_bass_kernel_spmd

    _log("host prep start")
    gp, lens, el_cores, u0_cores, Wf = _host_prep(logits, trans, labels, seq_lens)
    _log("host prep done")

    if "nc" not in _prog_cache:
        _prog_cache["nc"] = _build_program()
        _log("program built")
    nc = _prog_cache["nc"]

    in_maps = [
        {"el": el_cores[i], "u0": u0_cores[i], "wf": Wf}
        for i in range(NCORES)
    ]
    r = run_bass_kernel_spmd(nc, in_maps, core_ids=list(range(NCORES)))
    last_result = r
    _log("device run done")

    # ---- unshard: column sums of the saved states give both the scale
    # ratios and the captured Z (sink carries Z with actives dead, so the
    # column sum at each boundary tick is the single number needed) ----
    gsl = [(0, 43), (43, 86), (86, 128)]
    colE = np.zeros((C, B), np.float64)   # colsum at t = c*S + BURN - 1
    colF = np.zeros((C, B), np.float64)   # colsum at t = c*S + SP - 1 (= Z_j)
    for core in range(NCORES):
        uE = np.asarray(last_result.results[core]["uE"], np.float64)  # [99,COLS]
        uF = np.asarray(last_result.results[core]["uF"], np.float64)
        b0 = core * BPC
        for g, (s0, s1) in enumerate(gsl):
            nc_ = s1 - s0
            sE = uE[32 * g : 32 * g + 32].sum(0) + uE[NACT + g]
            sF = uF[32 * g : 32 * g + 32].sum(0) + uF[NACT + g]
            colE[:, b0 + s0 : b0 + s1] = sE.reshape(C, NCOL)[:, :nc_]
            colF[:, b0 + s0 : b0 + s1] = sF.reshape(C, NCOL)[:, :nc_]

    # ---- stitch scales: chain c valid for len in (c*S+BURN, c*S+SP] ----
    j = np.zeros(B, np.int64)
    for c in range(1, C):
        j[lens > c * S + BURN] = c
    with np.errstate(divide="ignore", invalid="ignore"):
        log_rho = np.log(colE[1:]) - np.log(colF[:-1])        # [C-1, B]
        log_gamma = np.concatenate(
            [np.zeros((1, B)), np.cumsum(log_rho, axis=0)], axis=0
        )                                                      # [C, B]
        log_sink = np.log(colF[j, np.arange(B)])
    logZ = log_sink - log_gamma[j, np.arange(B)] + CSHIFT * lens
    return (gp - logZ).astype(np.float32)


# revision 12
# speedup vs baseline: 1.6464x; 1.2185x over previous
"""CRF log-likelihood kernel for Trainium2 (Bass/Tile), 8-core data parallel.

out[b] = gold_path_score(b) - logZ(b)

logZ via chunked cold-start forward chains (exp domain).  The transfer
operator D_{el_t} Wf^T is strictly positive, so it contracts directions at
~0.3/step (Birkhoff): a chain started from the uniform vector forgets its
start after ~10 steps.  Split t = 1..T into C chunks of S steps; chain c
starts cold at t = c*S and runs BURN warm-up ticks + S valid ticks, all C
chains advancing in lockstep columns of the same matmul.  Serial depth drops
from T to SP = BURN + S ticks.

Scale recovery: chain c's state is gamma_c * alpha_t (unknown scalar
gamma_c).  At boundary time c*S + SP - 1 both chain c (final tick) and chain
c+1 (tick BURN) hold the same alpha direction, so the ratio of their Wf
column sums gives gamma_{c+1}/gamma_c exactly.  Sink rows capture
sum(alpha_{len-1}) at t == len per sequence (el32 gating), persist to the
chain end, and are read from the final tick's PSUM.  Host stitches:
logZ = log(sink_j) - log(gamma_j) + CSHIFT*len,  j = chunk containing len.

Per-core layout (128 sequences): partitions 0..95 = 3 label groups x 32,
96..98 = per-group sink rows; psum rows 99..101 = per-group column sums.
Columns: chain c occupies cols [43c, 43c+43); within a column, the 3 label
groups hold 3 different sequences (43+43+42+pad = 128).  Two column groups
of 10 chains each give two independent MM->TT streams that overlap on
PE/DVE.  Emissions exp'd on host, shipped bf16 (4x less DMA than f32
logits; el rounding is ~0.4% -> ~0.1 absolute logZ noise over 512 steps).
"""

import numpy as np
import ml_dtypes

B, T, L = 1024, 512, 32
NCORES = 8
BPC = B // NCORES        # 128 sequences per core
NCOL = 43                # columns per chain (3 label groups: 43+43+42+pad)
NACT = 96                # active label partitions
NPART = 99               # + 3 sink rows
MOUT = 102               # + 3 colsum rows
CSHIFT = 4.5
C = 30                   # chains (chunks)
BURN = 2                 # cold-start warm-up ticks
S = (T - BURN) // C      # valid ticks per chain (17)
assert BURN + C * S == T
SP = BURN + S            # ticks per chain (19)
COLS = NCOL * C          # 1290 total columns
GCOLS = 10 * NCOL        # three groups of 10 chains (430 cols)

_prog_cache = {}
last_result = None       # BassKernelResults of the most recent run (for test.py)


def _build_program():
    import concourse.bacc as bacc
    import concourse.tile as tile
    from concourse import mybir

    f32 = mybir.dt.float32
    bf16 = mybir.dt.bfloat16
    AF = mybir.ActivationFunctionType

    nc = bacc.Bacc("TRN2", target_bir_lowering=False, debug=False, num_devices=NCORES)
    el_d = nc.dram_tensor("el", [NPART, SP - 1, COLS], bf16, kind="ExternalInput")
    u0_d = nc.dram_tensor("u0", [NPART, NCOL], bf16, kind="ExternalInput")
    wf_d = nc.dram_tensor("wf", [NPART, MOUT], bf16, kind="ExternalInput")
    uE_d = nc.dram_tensor("uE", [NPART, COLS], bf16, kind="ExternalOutput")
    uF_d = nc.dram_tensor("uF", [NPART, COLS], bf16, kind="ExternalOutput")

    with tile.TileContext(nc) as tc:
        with (
            tc.tile_pool(name="consts", bufs=1) as consts,
            tc.tile_pool(name="elp", bufs=1) as elp,
            tc.tile_pool(name="uA", bufs=3) as uApool,
            tc.tile_pool(name="uB", bufs=3) as uBpool,
            tc.tile_pool(name="uC", bufs=3) as uCpool,
            tc.tile_pool(name="fin", bufs=1) as fin,
            tc.tile_pool(name="psA", bufs=2, space="PSUM") as psApool,
            tc.tile_pool(name="psB", bufs=2, space="PSUM") as psBpool,
            tc.tile_pool(name="psC", bufs=2, space="PSUM") as psCpool,
        ):
            wf_sb = consts.tile([NPART, MOUT], bf16)
            u0_sb = consts.tile([NPART, COLS], bf16)
            el_sb = elp.tile([NPART, SP - 1, COLS], bf16)
            uE_sb = fin.tile([NPART, COLS], bf16)

            # cold-start state is uniform 1 (sinks 0) everywhere except the
            # chain-0 block, which is DMA'd (= alpha_0); memsets keep the big
            # u0 off the DMA critical path.
            nc.vector.memset(u0_sb[:], 1.0)
            nc.vector.memset(u0_sb[NACT:NPART, :], 0.0)
            # wf + u0 block gate the first MM: sync queue, first.
            nc.sync.dma_start(out=wf_sb[:], in_=wf_d[:])
            nc.sync.dma_start(out=u0_sb[:, 0:NCOL], in_=u0_d[:])
            # el streams in tick-major chunks on the sync queue (the one
            # with the full DMA engine fan-out); first chunks small so the
            # chain starts early.
            edges = [0, 1, 2, 3, 4, 5, 6, 8, 10, 12, 14, 16, SP - 1]
            for k0, k1 in zip(edges[:-1], edges[1:]):
                nc.sync.dma_start(
                    out=el_sb[:, k0:k1, :], in_=el_d[:, k0:k1, :]
                )

            groups = [
                (uApool, psApool, 0, GCOLS),
                (uBpool, psBpool, GCOLS, 2 * GCOLS),
                (uCpool, psCpool, 2 * GCOLS, COLS),
            ]
            # ticks 1..SP-1; the boundary column sums (for scale stitching)
            # are recovered on the host from the raw states at tick BURN-1
            # (saved via an off-path ACT copy) and tick SP-1 (the final
            # state): captured-Z == column sum at the capture tick, so the
            # state tiles carry everything.
            uprev = [u0_sb[:, c0:c1] for (_, _, c0, c1) in groups]
            for k in range(1, SP):
                for gi, (upool, pspool, c0, c1) in enumerate(groups):
                    ps = pspool.tile([MOUT, c1 - c0], f32, tag=f"ps{gi}")
                    nc.tensor.matmul(ps[:], wf_sb[:], uprev[gi], start=True, stop=True)
                    un = upool.tile([NPART, c1 - c0], bf16, tag=f"u{gi}")
                    nc.vector.tensor_mul(
                        un[:], ps[0:NPART, :], el_sb[:, k - 1, c0:c1]
                    )
                    uprev[gi] = un[:]
                    if k == BURN - 1:
                        nc.scalar.activation(
                            uE_sb[:, c0:c1], un[:], AF.Copy
                        )
            for gi, (_, _, c0, c1) in enumerate(groups):
                nc.sync.dma_start(out=uF_d[:, c0:c1], in_=uprev[gi])
            nc.sync.dma_start(out=uE_d[:], in_=uE_sb[:])

    nc.compile()
    return nc


def _host_prep(logits, trans, labels, seq_lens):
    logits = np.ascontiguousarray(np.asarray(logits), dtype=np.float32)
    trans = np.asarray(trans, dtype=np.float32)
    labels = np.asarray(labels)
    lens = np.clip(np.asarray(seq_lens), 1, T).astype(np.int64)

    # ---- gold path score (host: index gathers over small inputs) ----
    tmask = np.arange(T)[None, :] < lens[:, None]
    unary = np.take_along_axis(logits, labels[..., None].astype(np.int64), axis=2)[..., 0]
    gp = (unary * tmask).sum(1) + (trans[labels[:, :-1], labels[:, 1:]] * tmask[:, 1:]).sum(1)

    # ---- emissions: exp on host, masked past seq end; slice t=T is
    # capture-only (el=0 everywhere, el32=1) ----
    lgx = logits.copy()
    lgx[~tmask] = -np.inf
    el_full = np.exp(lgx - CSHIFT)                                   # [B,T,L]
    el_full = np.concatenate([el_full, np.zeros((B, 1, L), np.float32)], axis=1)
    el32 = (np.arange(T + 1)[None, :] >= lens[:, None]).astype(np.float32)  # [B,T+1]

    bf = ml_dtypes.bfloat16
    gsl = [(0, 43), (43, 86), (86, 128)]  # local seq ranges per label group
    el_cores, u0_cores = [], []
    for core in range(NCORES):
        b0 = core * BPC
        E = el_full[b0 : b0 + BPC]          # [128, T+1, L]
        E32 = el32[b0 : b0 + BPC]           # [128, T+1]
        packed = np.zeros((NPART, SP - 1, COLS), np.float32)
        u0 = np.zeros((NPART, NCOL), np.float32)
        for c in range(C):
            t0 = c * S
            sl = E[:, t0 + 1 : t0 + SP, :]     # [128, SP-1, L]
            sl32 = E32[:, t0 + 1 : t0 + SP]    # [128, SP-1]
            for g, (s0, s1) in enumerate(gsl):
                nc_ = s1 - s0
                cc = NCOL * c
                packed[32 * g : 32 * g + 32, :, cc : cc + nc_] = sl[s0:s1].transpose(2, 1, 0)
                packed[NACT + g, :, cc : cc + nc_] = sl32[s0:s1].T
                if c == 0:
                    u0[32 * g : 32 * g + 32, :nc_] = E[s0:s1, 0, :].T
        el_cores.append(packed.astype(bf))
        u0_cores.append(u0.astype(bf))

    # ---- stationary operator: block-diag exp(trans) + sink + colsum ----
    Ew = np.exp(trans).astype(np.float32)
    Wf = np.zeros((NPART, MOUT), np.float32)
    for g in range(3):
        a, sk, cs = 32 * g, NACT + g, NPART + g
        Wf[a : a + 32, a : a + 32] = Ew
        Wf[a : a + 32, sk] = 1.0
        Wf[sk, sk] = 1.0
        Wf[a : a + 32, cs] = 1.0
        Wf[sk, cs] = 1.0
    return gp, lens, el_cores, u0_cores, Wf.astype(bf)


def _log(msg):
    import time as _t

    print(f"[kernel {_t.strftime('%H:%M:%S')}] {msg}", flush=True)


def kernel(logits, trans, labels, seq_lens):
    global last_result
    from concourse.bass_utils import run_bass_kernel_spmd

    _log("host prep start")
    gp, lens, el_cores, u0_cores, Wf = _host_prep(logits, trans, labels, seq_lens)
    _log("host prep done")

    if "nc" not in _prog_cache:
        _prog_cache["nc"] = _build_program()
        _log("program built")
    nc = _prog_cache["nc"]

    in_maps = [
        {"el": el_cores[i], "u0": u0_cores[i], "wf": Wf}
        for i in range(NCORES)
    ]
    r = run_bass_kernel_spmd(nc, in_maps, core_ids=list(range(NCORES)))
    last_result = r
    _log("device run done")

    # ---- unshard: column sums of the saved states give both the scale
    # ratios and the captured Z (sink carries Z with actives dead, so the
    # column sum at each boundary tick is the single number needed) ----
    gsl = [(0, 43), (43, 86), (86, 128)]
    colE = np.zeros((C, B), np.float64)   # colsum at t = c*S + BURN - 1
    colF = np.zeros((C, B), np.float64)   # colsum at t = c*S + SP - 1 (= Z_j)
    for core in range(NCORES):
        uE = np.asarray(last_result.results[core]["uE"], np.float64)  # [99,COLS]
        uF = np.asarray(last_result.results[core]["uF"], np.float64)
        b0 = core * BPC
        for g, (s0, s1) in enumerate(gsl):
            nc_ = s1 - s0
            sE = uE[32 * g : 32 * g + 32].sum(0) + uE[NACT + g]
            sF = uF[32 * g : 32 * g + 32].sum(0) + uF[NACT + g]
            colE[:, b0 + s0 : b0 + s1] = sE.reshape(C, NCOL)[:, :nc_]
            colF[:, b0 + s0 : b0 + s1] = sF.reshape(C, NCOL)[:, :nc_]

    # ---- stitch scales: chain c valid for len in (c*S+BURN, c*S+SP] ----
    j = np.zeros(B, np.int64)
    for c in range(1, C):
        j[lens > c * S + BURN] = c
    with np.errstate(divide="ignore", invalid="ignore"):
        log_rho = np.log(colE[1:]) - np.log(colF[:-1])        # [C-1, B]
        log_gamma = np.concatenate(
            [np.zeros((1, B)), np.cumsum(log_rho, axis=0)], axis=0
        )                                                      # [C, B]
        log_sink = np.log(colF[j, np.arange(B)])
    logZ = log_sink - log_gamma[j, np.arange(B)] + CSHIFT * lens
    return (gp - logZ).astype(np.float32)


# revision 13
# speedup vs baseline: 1.6485x; 1.0013x over previous
"""CRF log-likelihood kernel for Trainium2 (Bass/Tile), 8-core data parallel.

out[b] = gold_path_score(b) - logZ(b)

logZ via chunked cold-start forward chains (exp domain).  The transfer
operator D_{el_t} Wf^T is strictly positive, so it contracts directions at
~0.3/step (Birkhoff): a chain started from the uniform vector forgets its
start after ~10 steps.  Split t = 1..T into C chunks of S steps; chain c
starts cold at t = c*S and runs BURN warm-up ticks + S valid ticks, all C
chains advancing in lockstep columns of the same matmul.  Serial depth drops
from T to SP = BURN + S ticks.

Scale recovery: chain c's state is gamma_c * alpha_t (unknown scalar
gamma_c).  At boundary time c*S + SP - 1 both chain c (final tick) and chain
c+1 (tick BURN) hold the same alpha direction, so the ratio of their Wf
column sums gives gamma_{c+1}/gamma_c exactly.  Sink rows capture
sum(alpha_{len-1}) at t == len per sequence (el32 gating), persist to the
chain end, and are read from the final tick's PSUM.  Host stitches:
logZ = log(sink_j) - log(gamma_j) + CSHIFT*len,  j = chunk containing len.

Per-core layout (128 sequences): partitions 0..95 = 3 label groups x 32,
96..98 = per-group sink rows; psum rows 99..101 = per-group column sums.
Columns: chain c occupies cols [43c, 43c+43); within a column, the 3 label
groups hold 3 different sequences (43+43+42+pad = 128).  Two column groups
of 10 chains each give two independent MM->TT streams that overlap on
PE/DVE.  Emissions exp'd on host, shipped bf16 (4x less DMA than f32
logits; el rounding is ~0.4% -> ~0.1 absolute logZ noise over 512 steps).
"""

import numpy as np
import ml_dtypes

B, T, L = 1024, 512, 32
NCORES = 8
BPC = B // NCORES        # 128 sequences per core
NCOL = 43                # columns per chain (3 label groups: 43+43+42+pad)
NACT = 96                # active label partitions
NPART = 99               # + 3 sink rows
MOUT = 102               # + 3 colsum rows
CSHIFT = 4.5
C = 30                   # chains (chunks)
BURN = 2                 # cold-start warm-up ticks
S = (T - BURN) // C      # valid ticks per chain (17)
assert BURN + C * S == T
SP = BURN + S            # ticks per chain (19)
COLS = NCOL * C          # 1290 total columns
GCOLS = 10 * NCOL        # three groups of 10 chains (430 cols)

_prog_cache = {}
last_result = None       # BassKernelResults of the most recent run (for test.py)


def _build_program():
    import concourse.bacc as bacc
    import concourse.tile as tile
    from concourse import mybir

    f32 = mybir.dt.float32
    bf16 = mybir.dt.bfloat16
    AF = mybir.ActivationFunctionType

    nc = bacc.Bacc("TRN2", target_bir_lowering=False, debug=False, num_devices=NCORES)
    el_d = nc.dram_tensor("el", [NPART, SP - 1, COLS], bf16, kind="ExternalInput")
    u0_d = nc.dram_tensor("u0", [NPART, NCOL], bf16, kind="ExternalInput")
    wf_d = nc.dram_tensor("wf", [NPART, MOUT], bf16, kind="ExternalInput")
    uE_d = nc.dram_tensor("uE", [NPART, COLS], bf16, kind="ExternalOutput")
    uF_d = nc.dram_tensor("uF", [NPART, COLS], bf16, kind="ExternalOutput")

    with tile.TileContext(nc) as tc:
        with (
            tc.tile_pool(name="consts", bufs=1) as consts,
            tc.tile_pool(name="elp", bufs=1) as elp,
            tc.tile_pool(name="uA", bufs=3) as uApool,
            tc.tile_pool(name="uB", bufs=3) as uBpool,
            tc.tile_pool(name="uC", bufs=3) as uCpool,
            tc.tile_pool(name="fin", bufs=1) as fin,
            tc.tile_pool(name="psA", bufs=2, space="PSUM") as psApool,
            tc.tile_pool(name="psB", bufs=2, space="PSUM") as psBpool,
            tc.tile_pool(name="psC", bufs=2, space="PSUM") as psCpool,
        ):
            wf_sb = consts.tile([NPART, MOUT], bf16)
            u0_sb = consts.tile([NPART, COLS], bf16)
            el_sb = elp.tile([NPART, SP - 1, COLS], bf16)
            uE_sb = fin.tile([NPART, COLS], bf16)

            # cold-start state is uniform 1 (sinks 0) everywhere except the
            # chain-0 block, which is DMA'd (= alpha_0); memsets keep the big
            # u0 off the DMA critical path.
            nc.vector.memset(u0_sb[:], 1.0)
            nc.vector.memset(u0_sb[NACT:NPART, :], 0.0)
            # wf + u0 block gate the first MM: tiny transfers, issued on
            # the idle gpsimd/scalar queues so the sync queue starts el
            # immediately.
            nc.gpsimd.dma_start(out=wf_sb[:], in_=wf_d[:])
            nc.scalar.dma_start(out=u0_sb[:, 0:NCOL], in_=u0_d[:])
            # el streams in tick-major chunks on the sync queue (the one
            # with the full DMA engine fan-out); first chunks small so the
            # chain starts early.
            edges = [0, 1, 2, 3, 4, 5, 6, 8, 10, 12, 14, 16, SP - 1]
            for k0, k1 in zip(edges[:-1], edges[1:]):
                nc.sync.dma_start(
                    out=el_sb[:, k0:k1, :], in_=el_d[:, k0:k1, :]
                )

            groups = [
                (uApool, psApool, 0, GCOLS),
                (uBpool, psBpool, GCOLS, 2 * GCOLS),
                (uCpool, psCpool, 2 * GCOLS, COLS),
            ]
            # ticks 1..SP-1; the boundary column sums (for scale stitching)
            # are recovered on the host from the raw states at tick BURN-1
            # (saved via an off-path ACT copy) and tick SP-1 (the final
            # state): captured-Z == column sum at the capture tick, so the
            # state tiles carry everything.
            uprev = [u0_sb[:, c0:c1] for (_, _, c0, c1) in groups]
            for k in range(1, SP):
                for gi, (upool, pspool, c0, c1) in enumerate(groups):
                    ps = pspool.tile([MOUT, c1 - c0], f32, tag=f"ps{gi}")
                    nc.tensor.matmul(ps[:], wf_sb[:], uprev[gi], start=True, stop=True)
                    un = upool.tile([NPART, c1 - c0], bf16, tag=f"u{gi}")
                    nc.vector.tensor_mul(
                        un[:], ps[0:NPART, :], el_sb[:, k - 1, c0:c1]
                    )
                    uprev[gi] = un[:]
                    if k == BURN - 1:
                        nc.scalar.activation(
                            uE_sb[:, c0:c1], un[:], AF.Copy
                        )
                    if k == BURN:
                        nc.gpsimd.dma_start(
                            out=uE_d[:, c0:c1], in_=uE_sb[:, c0:c1]
                        )
                    if k == SP - 1:
                        nc.sync.dma_start(out=uF_d[:, c0:c1], in_=un[:])

    nc.compile()
    return nc


def _host_prep(logits, trans, labels, seq_lens):
    logits = np.ascontiguousarray(np.asarray(logits), dtype=np.float32)
    trans = np.asarray(trans, dtype=np.float32)
    labels = np.asarray(labels)
    lens = np.clip(np.asarray(seq_lens), 1, T).astype(np.int64)

    # ---- gold path score (host: index gathers over small inputs) ----
    tmask = np.arange(T)[None, :] < lens[:, None]
    unary = np.take_along_axis(logits, labels[..., None].astype(np.int64), axis=2)[..., 0]
    gp = (unary * tmask).sum(1) + (trans[labels[:, :-1], labels[:, 1:]] * tmask[:, 1:]).sum(1)

    # ---- emissions: exp on host, masked past seq end; slice t=T is
    # capture-only (el=0 everywhere, el32=1) ----
    lgx = logits.copy()
    lgx[~tmask] = -np.inf
    el_full = np.exp(lgx - CSHIFT)                                   # [B,T,L]
    el_full = np.concatenate([el_full, np.zeros((B, 1, L), np.float32)], axis=1)
    el32 = (np.arange(T + 1)[None, :] >= lens[:, None]).astype(np.float32)  # [B,T+1]

    bf = ml_dtypes.bfloat16
    gsl = [(0, 43), (43, 86), (86, 128)]  # local seq ranges per label group
    el_cores, u0_cores = [], []
    for core in range(NCORES):
        b0 = core * BPC
        E = el_full[b0 : b0 + BPC]          # [128, T+1, L]
        E32 = el32[b0 : b0 + BPC]           # [128, T+1]
        packed = np.zeros((NPART, SP - 1, COLS), np.float32)
        u0 = np.zeros((NPART, NCOL), np.float32)
        for c in range(C):
            t0 = c * S
            sl = E[:, t0 + 1 : t0 + SP, :]     # [128, SP-1, L]
            sl32 = E32[:, t0 + 1 : t0 + SP]    # [128, SP-1]
            for g, (s0, s1) in enumerate(gsl):
                nc_ = s1 - s0
                cc = NCOL * c
                packed[32 * g : 32 * g + 32, :, cc : cc + nc_] = sl[s0:s1].transpose(2, 1, 0)
                packed[NACT + g, :, cc : cc + nc_] = sl32[s0:s1].T
                if c == 0:
                    u0[32 * g : 32 * g + 32, :nc_] = E[s0:s1, 0, :].T
        el_cores.append(packed.astype(bf))
        u0_cores.append(u0.astype(bf))

    # ---- stationary operator: block-diag exp(trans) + sink + colsum ----
    Ew = np.exp(trans).astype(np.float32)
    Wf = np.zeros((NPART, MOUT), np.float32)
    for g in range(3):
        a, sk, cs = 32 * g, NACT + g, NPART + g
        Wf[a : a + 32, a : a + 32] = Ew
        Wf[a : a + 32, sk] = 1.0
        Wf[sk, sk] = 1.0
        Wf[a : a + 32, cs] = 1.0
        Wf[sk, cs] = 1.0
    return gp, lens, el_cores, u0_cores, Wf.astype(bf)


def _log(msg):
    import time as _t

    print(f"[kernel {_t.strftime('%H:%M:%S')}] {msg}", flush=True)


def kernel(logits, trans, labels, seq_lens):
    global last_result
    from concourse.bass_utils import run_bass_kernel_spmd

    _log("host prep start")
    gp, lens, el_cores, u0_cores, Wf = _host_prep(logits, trans, labels, seq_lens)
    _log("host prep done")

    if "nc" not in _prog_cache:
        _prog_cache["nc"] = _build_program()
        _log("program built")
    nc = _prog_cache["nc"]

    in_maps = [
        {"el": el_cores[i], "u0": u0_cores[i], "wf": Wf}
        for i in range(NCORES)
    ]
    r = run_bass_kernel_spmd(nc, in_maps, core_ids=list(range(NCORES)))
    last_result = r
    _log("device run done")

    # ---- unshard: column sums of the saved states give both the scale
    # ratios and the captured Z (sink carries Z with actives dead, so the
    # column sum at each boundary tick is the single number needed) ----
    gsl = [(0, 43), (43, 86), (86, 128)]
    colE = np.zeros((C, B), np.float64)   # colsum at t = c*S + BURN - 1
    colF = np.zeros((C, B), np.float64)   # colsum at t = c*S + SP - 1 (= Z_j)
    for core in range(NCORES):
        uE = np.asarray(last_result.results[core]["uE"], np.float64)  # [99,COLS]
        uF = np.asarray(last_result.results[core]["uF"], np.float64)
        b0 = core * BPC
        for g, (s0, s1) in enumerate(gsl):
            nc_ = s1 - s0
            sE = uE[32 * g : 32 * g + 32].sum(0) + uE[NACT + g]
            sF = uF[32 * g : 32 * g + 32].sum(0) + uF[NACT + g]
            colE[:, b0 + s0 : b0 + s1] = sE.reshape(C, NCOL)[:, :nc_]
            colF[:, b0 + s0 : b0 + s1] = sF.reshape(C, NCOL)[:, :nc_]

    # ---- stitch scales: chain c valid for len in (c*S+BURN, c*S+SP] ----
    j = np.zeros(B, np.int64)
    for c in range(1, C):
        j[lens > c * S + BURN] = c
    with np.errstate(divide="ignore", invalid="ignore"):
        log_rho = np.log(colE[1:]) - np.log(colF[:-1])        # [C-1, B]
        log_gamma = np.concatenate(
            [np.zeros((1, B)), np.cumsum(log_rho, axis=0)], axis=0
        )                                                      # [C, B]
        log_sink = np.log(colF[j, np.arange(B)])
    logZ = log_sink - log_gamma[j, np.arange(B)] + CSHIFT * lens
    return (gp - logZ).astype(np.float32)


# revision 14
# speedup vs baseline: 1.7070x; 1.0355x over previous
"""CRF log-likelihood kernel for Trainium2 (Bass/Tile), 8-core data parallel.

out[b] = gold_path_score(b) - logZ(b)

logZ via chunked cold-start forward chains (exp domain).  The transfer
operator D_{el_t} Wf^T is strictly positive, so it contracts directions at
~0.3/step (Birkhoff): a chain started from the uniform vector forgets its
start after ~10 steps.  Split t = 1..T into C chunks of S steps; chain c
starts cold at t = c*S and runs BURN warm-up ticks + S valid ticks, all C
chains advancing in lockstep columns of the same matmul.  Serial depth drops
from T to SP = BURN + S ticks.

Scale recovery: chain c's state is gamma_c * alpha_t (unknown scalar
gamma_c).  At boundary time c*S + SP - 1 both chain c (final tick) and chain
c+1 (tick BURN) hold the same alpha direction, so the ratio of their Wf
column sums gives gamma_{c+1}/gamma_c exactly.  Sink rows capture
sum(alpha_{len-1}) at t == len per sequence (el32 gating), persist to the
chain end, and are read from the final tick's PSUM.  Host stitches:
logZ = log(sink_j) - log(gamma_j) + CSHIFT*len,  j = chunk containing len.

Per-core layout (128 sequences): partitions 0..95 = 3 label groups x 32,
96..98 = per-group sink rows; psum rows 99..101 = per-group column sums.
Columns: chain c occupies cols [43c, 43c+43); within a column, the 3 label
groups hold 3 different sequences (43+43+42+pad = 128).  Two column groups
of 10 chains each give two independent MM->TT streams that overlap on
PE/DVE.  Emissions exp'd on host, shipped bf16 (4x less DMA than f32
logits; el rounding is ~0.4% -> ~0.1 absolute logZ noise over 512 steps).
"""

import numpy as np
import ml_dtypes

B, T, L = 1024, 512, 32
NCORES = 8
BPC = B // NCORES        # 128 sequences per core
NCOL = 43                # columns per chain (3 label groups: 43+43+42+pad)
NACT = 96                # active label partitions
NPART = 99               # + 3 sink rows
MOUT = 102               # + 3 colsum rows
CSHIFT = 4.5
C = 30                   # chains (chunks)
BURN = 2                 # cold-start warm-up ticks
S = (T - BURN) // C      # valid ticks per chain (17)
assert BURN + C * S == T
SP = BURN + S            # ticks per chain (19)
COLS = NCOL * C          # 1290 total columns
GCOLS = 10 * NCOL        # three groups of 10 chains (430 cols)

_prog_cache = {}
last_result = None       # BassKernelResults of the most recent run (for test.py)


def _build_program():
    import concourse.bacc as bacc
    import concourse.tile as tile
    from concourse import mybir

    f32 = mybir.dt.float32
    bf16 = mybir.dt.bfloat16
    AF = mybir.ActivationFunctionType

    nc = bacc.Bacc("TRN2", target_bir_lowering=False, debug=False, num_devices=NCORES)
    el_d = nc.dram_tensor("el", [NPART, SP - 1, COLS], bf16, kind="ExternalInput")
    u0_d = nc.dram_tensor("u0", [NPART, COLS], bf16, kind="ExternalInput")
    wf_d = nc.dram_tensor("wf", [NPART, MOUT], bf16, kind="ExternalInput")
    uE_d = nc.dram_tensor("uE", [NPART, COLS], bf16, kind="ExternalOutput")
    uF_d = nc.dram_tensor("uF", [NPART, COLS], bf16, kind="ExternalOutput")

    with tile.TileContext(nc) as tc:
        with (
            tc.tile_pool(name="consts", bufs=1) as consts,
            tc.tile_pool(name="elp", bufs=1) as elp,
            tc.tile_pool(name="uA", bufs=3) as uApool,
            tc.tile_pool(name="uB", bufs=3) as uBpool,
            tc.tile_pool(name="uC", bufs=3) as uCpool,
            tc.tile_pool(name="fin", bufs=1) as fin,
            tc.tile_pool(name="psA", bufs=2, space="PSUM") as psApool,
            tc.tile_pool(name="psB", bufs=2, space="PSUM") as psBpool,
            tc.tile_pool(name="psC", bufs=2, space="PSUM") as psCpool,
        ):
            wf_sb = consts.tile([NPART, MOUT], bf16)
            u0_sb = consts.tile([NPART, COLS], bf16)
            el_sb = elp.tile([NPART, SP - 1, COLS], bf16)
            uE_sb = fin.tile([NPART, COLS], bf16)

            # u0 + wf gate the first MM: first on the sync queue, ahead of
            # the el stream.
            nc.sync.dma_start(out=u0_sb[:], in_=u0_d[:])
            nc.sync.dma_start(out=wf_sb[:], in_=wf_d[:])
            # el streams in tick-major chunks on the sync queue (the one
            # with the full DMA engine fan-out); first chunks small so the
            # chain starts early.
            edges = [0, 1, 2, 3, 4, 5, 6, 8, 10, 12, 14, 16, SP - 1]
            for k0, k1 in zip(edges[:-1], edges[1:]):
                nc.sync.dma_start(
                    out=el_sb[:, k0:k1, :], in_=el_d[:, k0:k1, :]
                )

            groups = [
                (uApool, psApool, 0, GCOLS),
                (uBpool, psBpool, GCOLS, 2 * GCOLS),
                (uCpool, psCpool, 2 * GCOLS, COLS),
            ]
            # ticks 1..SP-1; the boundary column sums (for scale stitching)
            # are recovered on the host from the raw states at tick BURN-1
            # (saved via an off-path ACT copy) and tick SP-1 (the final
            # state): captured-Z == column sum at the capture tick, so the
            # state tiles carry everything.
            uprev = [u0_sb[:, c0:c1] for (_, _, c0, c1) in groups]
            for k in range(1, SP):
                for gi, (upool, pspool, c0, c1) in enumerate(groups):
                    ps = pspool.tile([MOUT, c1 - c0], f32, tag=f"ps{gi}")
                    nc.tensor.matmul(ps[:], wf_sb[:], uprev[gi], start=True, stop=True)
                    un = upool.tile([NPART, c1 - c0], bf16, tag=f"u{gi}")
                    nc.vector.tensor_mul(
                        un[:], ps[0:NPART, :], el_sb[:, k - 1, c0:c1]
                    )
                    uprev[gi] = un[:]
                    if k == BURN - 1:
                        nc.scalar.activation(
                            uE_sb[:, c0:c1], un[:], AF.Copy
                        )
                    if k == BURN:
                        nc.gpsimd.dma_start(
                            out=uE_d[:, c0:c1], in_=uE_sb[:, c0:c1]
                        )
                    if k == SP - 1:
                        # one output DMA per group on separate queues, each
                        # issued as soon as that group's last TT lands.
                        [nc.gpsimd, nc.scalar, nc.sync][gi].dma_start(
                            out=uF_d[:, c0:c1], in_=un[:]
                        )

    nc.compile()
    return nc


def _host_prep(logits, trans, labels, seq_lens):
    logits = np.ascontiguousarray(np.asarray(logits), dtype=np.float32)
    trans = np.asarray(trans, dtype=np.float32)
    labels = np.asarray(labels)
    lens = np.clip(np.asarray(seq_lens), 1, T).astype(np.int64)

    # ---- gold path score (host: index gathers over small inputs) ----
    tmask = np.arange(T)[None, :] < lens[:, None]
    unary = np.take_along_axis(logits, labels[..., None].astype(np.int64), axis=2)[..., 0]
    gp = (unary * tmask).sum(1) + (trans[labels[:, :-1], labels[:, 1:]] * tmask[:, 1:]).sum(1)

    # ---- emissions: exp on host, masked past seq end; slice t=T is
    # capture-only (el=0 everywhere, el32=1) ----
    lgx = logits.copy()
    lgx[~tmask] = -np.inf
    el_full = np.exp(lgx - CSHIFT)                                   # [B,T,L]
    el_full = np.concatenate([el_full, np.zeros((B, 1, L), np.float32)], axis=1)
    el32 = (np.arange(T + 1)[None, :] >= lens[:, None]).astype(np.float32)  # [B,T+1]

    bf = ml_dtypes.bfloat16
    gsl = [(0, 43), (43, 86), (86, 128)]  # local seq ranges per label group
    el_cores, u0_cores = [], []
    for core in range(NCORES):
        b0 = core * BPC
        E = el_full[b0 : b0 + BPC]          # [128, T+1, L]
        E32 = el32[b0 : b0 + BPC]           # [128, T+1]
        packed = np.zeros((NPART, SP - 1, COLS), np.float32)
        u0 = np.ones((NPART, COLS), np.float32)
        u0[NACT:NPART] = 0.0
        for c in range(C):
            t0 = c * S
            sl = E[:, t0 + 1 : t0 + SP, :]     # [128, SP-1, L]
            sl32 = E32[:, t0 + 1 : t0 + SP]    # [128, SP-1]
            for g, (s0, s1) in enumerate(gsl):
                nc_ = s1 - s0
                cc = NCOL * c
                packed[32 * g : 32 * g + 32, :, cc : cc + nc_] = sl[s0:s1].transpose(2, 1, 0)
                packed[NACT + g, :, cc : cc + nc_] = sl32[s0:s1].T
                if c == 0:
                    u0[32 * g : 32 * g + 32, cc : cc + nc_] = E[s0:s1, 0, :].T
                    if nc_ < NCOL:
                        u0[32 * g : 32 * g + 32, cc + nc_ : cc + NCOL] = 0.0
        el_cores.append(packed.astype(bf))
        u0_cores.append(u0.astype(bf))

    # ---- stationary operator: block-diag exp(trans) + sink + colsum ----
    Ew = np.exp(trans).astype(np.float32)
    Wf = np.zeros((NPART, MOUT), np.float32)
    for g in range(3):
        a, sk, cs = 32 * g, NACT + g, NPART + g
        Wf[a : a + 32, a : a + 32] = Ew
        Wf[a : a + 32, sk] = 1.0
        Wf[sk, sk] = 1.0
        Wf[a : a + 32, cs] = 1.0
        Wf[sk, cs] = 1.0
    return gp, lens, el_cores, u0_cores, Wf.astype(bf)


def _log(msg):
    import time as _t

    print(f"[kernel {_t.strftime('%H:%M:%S')}] {msg}", flush=True)


def kernel(logits, trans, labels, seq_lens):
    global last_result
    from concourse.bass_utils import run_bass_kernel_spmd

    _log("host prep start")
    gp, lens, el_cores, u0_cores, Wf = _host_prep(logits, trans, labels, seq_lens)
    _log("host prep done")

    if "nc" not in _prog_cache:
        _prog_cache["nc"] = _build_program()
        _log("program built")
    nc = _prog_cache["nc"]

    in_maps = [
        {"el": el_cores[i], "u0": u0_cores[i], "wf": Wf}
        for i in range(NCORES)
    ]
    r = run_bass_kernel_spmd(nc, in_maps, core_ids=list(range(NCORES)))
    last_result = r
    _log("device run done")

    # ---- unshard: column sums of the saved states give both the scale
    # ratios and the captured Z (sink carries Z with actives dead, so the
    # column sum at each boundary tick is the single number needed) ----
    gsl = [(0, 43), (43, 86), (86, 128)]
    colE = np.zeros((C, B), np.float64)   # colsum at t = c*S + BURN - 1
    colF = np.zeros((C, B), np.float64)   # colsum at t = c*S + SP - 1 (= Z_j)
    for core in range(NCORES):
        uE = np.asarray(last_result.results[core]["uE"], np.float64)  # [99,COLS]
        uF = np.asarray(last_result.results[core]["uF"], np.float64)
        b0 = core * BPC
        for g, (s0, s1) in enumerate(gsl):
            nc_ = s1 - s0
            sE = uE[32 * g : 32 * g + 32].sum(0) + uE[NACT + g]
            sF = uF[32 * g : 32 * g + 32].sum(0) + uF[NACT + g]
            colE[:, b0 + s0 : b0 + s1] = sE.reshape(C, NCOL)[:, :nc_]
            colF[:, b0 + s0 : b0 + s1] = sF.reshape(C, NCOL)[:, :nc_]

    # ---- stitch scales: chain c valid for len in (c*S+BURN, c*S+SP] ----
    j = np.zeros(B, np.int64)
    for c in range(1, C):
        j[lens > c * S + BURN] = c
    with np.errstate(divide="ignore", invalid="ignore"):
        log_rho = np.log(colE[1:]) - np.log(colF[:-1])        # [C-1, B]
        log_gamma = np.concatenate(
            [np.zeros((1, B)), np.cumsum(log_rho, axis=0)], axis=0
        )                                                      # [C, B]
        log_sink = np.log(colF[j, np.arange(B)])
    logZ = log_sink - log_gamma[j, np.arange(B)] + CSHIFT * lens
    return (gp - logZ).astype(np.float32)


# revision 15
# speedup vs baseline: 1.7647x; 1.0338x over previous
"""CRF log-likelihood kernel for Trainium2 (Bass/Tile), 8-core data parallel.

out[b] = gold_path_score(b) - logZ(b)

logZ via chunked cold-start forward chains (exp domain).  The transfer
operator D_{el_t} Wf^T is strictly positive, so it contracts directions at
~0.3/step (Birkhoff): a chain started from the uniform vector forgets its
start after ~10 steps.  Split t = 1..T into C chunks of S steps; chain c
starts cold at t = c*S and runs BURN warm-up ticks + S valid ticks, all C
chains advancing in lockstep columns of the same matmul.  Serial depth drops
from T to SP = BURN + S ticks.

Scale recovery: chain c's state is gamma_c * alpha_t (unknown scalar
gamma_c).  At boundary time c*S + SP - 1 both chain c (final tick) and chain
c+1 (tick BURN) hold the same alpha direction, so the ratio of their Wf
column sums gives gamma_{c+1}/gamma_c exactly.  Sink rows capture
sum(alpha_{len-1}) at t == len per sequence (el32 gating), persist to the
chain end, and are read from the final tick's PSUM.  Host stitches:
logZ = log(sink_j) - log(gamma_j) + CSHIFT*len,  j = chunk containing len.

Per-core layout (128 sequences): partitions 0..95 = 3 label groups x 32,
96..98 = per-group sink rows; psum rows 99..101 = per-group column sums.
Columns: chain c occupies cols [43c, 43c+43); within a column, the 3 label
groups hold 3 different sequences (43+43+42+pad = 128).  Two column groups
of 10 chains each give two independent MM->TT streams that overlap on
PE/DVE.  Emissions exp'd on host, shipped bf16 (4x less DMA than f32
logits; el rounding is ~0.4% -> ~0.1 absolute logZ noise over 512 steps).
"""

import numpy as np
import ml_dtypes

B, T, L = 1024, 512, 32
NCORES = 8
BPC = B // NCORES        # 128 sequences per core
NCOL = 43                # columns per chain (3 label groups: 43+43+42+pad)
NACT = 96                # active label partitions
NPART = 99               # + 3 sink rows
MOUT = 102               # + 3 colsum rows
CSHIFT = 4.5
C = 30                   # chains (chunks)
BURN = 2                 # cold-start warm-up ticks
S = (T - BURN) // C      # valid ticks per chain (17)
assert BURN + C * S == T
SP = BURN + S            # ticks per chain (19)
COLS = NCOL * C          # 1290 total columns
GCOLS = 10 * NCOL        # three groups of 10 chains (430 cols)

_prog_cache = {}
last_result = None       # BassKernelResults of the most recent run (for test.py)


def _build_program():
    import concourse.bacc as bacc
    import concourse.tile as tile
    from concourse import mybir

    f32 = mybir.dt.float32
    bf16 = mybir.dt.bfloat16
    AF = mybir.ActivationFunctionType

    nc = bacc.Bacc("TRN2", target_bir_lowering=False, debug=False, num_devices=NCORES)
    el_d = nc.dram_tensor("el", [NPART, SP - 2, COLS], bf16, kind="ExternalInput")
    u0_d = nc.dram_tensor("u0", [NPART, COLS], bf16, kind="ExternalInput")
    wf_d = nc.dram_tensor("wf", [NPART, MOUT], bf16, kind="ExternalInput")
    uF_d = nc.dram_tensor("uF", [NPART, COLS], bf16, kind="ExternalOutput")

    with tile.TileContext(nc) as tc:
        with (
            tc.tile_pool(name="consts", bufs=1) as consts,
            tc.tile_pool(name="elp", bufs=1) as elp,
            tc.tile_pool(name="uA", bufs=3) as uApool,
            tc.tile_pool(name="uB", bufs=3) as uBpool,
            tc.tile_pool(name="uC", bufs=3) as uCpool,
            tc.tile_pool(name="fin", bufs=1) as fin,
            tc.tile_pool(name="psA", bufs=2, space="PSUM") as psApool,
            tc.tile_pool(name="psB", bufs=2, space="PSUM") as psBpool,
            tc.tile_pool(name="psC", bufs=2, space="PSUM") as psCpool,
        ):
            wf_sb = consts.tile([NPART, MOUT], bf16)
            u0_sb = consts.tile([NPART, COLS], bf16)
            el_sb = elp.tile([NPART, SP - 2, COLS], bf16)

            # u0 + wf gate the first MM: first on the sync queue, ahead of
            # the el stream.
            nc.sync.dma_start(out=u0_sb[:], in_=u0_d[:])
            nc.sync.dma_start(out=wf_sb[:], in_=wf_d[:])
            # el streams in tick-major chunks on the sync queue (the one
            # with the full DMA engine fan-out); first chunks small so the
            # chain starts early.
            edges = [0, 1, 2, 3, 4, 5, 6, 8, 10, 12, 14, SP - 2]
            for k0, k1 in zip(edges[:-1], edges[1:]):
                nc.sync.dma_start(
                    out=el_sb[:, k0:k1, :], in_=el_d[:, k0:k1, :]
                )

            groups = [
                (uApool, psApool, 0, GCOLS),
                (uBpool, psBpool, GCOLS, 2 * GCOLS),
                (uCpool, psCpool, 2 * GCOLS, COLS),
            ]
            # ticks 2..SP-1 (the host precomputes the tick-1 state u1 and
            # its column sums; with BURN=2 the early boundary snapshot IS
            # u1, so the device only captures the final state): captured-Z
            # == column sum at the capture tick, so uF carries everything.
            uprev = [u0_sb[:, c0:c1] for (_, _, c0, c1) in groups]
            for k in range(2, SP):
                for gi, (upool, pspool, c0, c1) in enumerate(groups):
                    ps = pspool.tile([MOUT, c1 - c0], f32, tag=f"ps{gi}")
                    nc.tensor.matmul(ps[:], wf_sb[:], uprev[gi], start=True, stop=True)
                    un = upool.tile([NPART, c1 - c0], bf16, tag=f"u{gi}")
                    nc.vector.tensor_mul(
                        un[:], ps[0:NPART, :], el_sb[:, k - 2, c0:c1]
                    )
                    uprev[gi] = un[:]
                    if k == SP - 1:
                        # one output DMA per group on separate queues, each
                        # issued as soon as that group's last TT lands.
                        [nc.gpsimd, nc.scalar, nc.sync][gi].dma_start(
                            out=uF_d[:, c0:c1], in_=un[:]
                        )

    nc.compile()
    return nc


def _host_prep(logits, trans, labels, seq_lens):
    logits = np.ascontiguousarray(np.asarray(logits), dtype=np.float32)
    trans = np.asarray(trans, dtype=np.float32)
    labels = np.asarray(labels)
    lens = np.clip(np.asarray(seq_lens), 1, T).astype(np.int64)

    # ---- gold path score (host: index gathers over small inputs) ----
    tmask = np.arange(T)[None, :] < lens[:, None]
    unary = np.take_along_axis(logits, labels[..., None].astype(np.int64), axis=2)[..., 0]
    gp = (unary * tmask).sum(1) + (trans[labels[:, :-1], labels[:, 1:]] * tmask[:, 1:]).sum(1)

    # ---- emissions: exp on host, masked past seq end; slice t=T is
    # capture-only (el=0 everywhere, el32=1) ----
    lgx = logits.copy()
    lgx[~tmask] = -np.inf
    el_full = np.exp(lgx - CSHIFT)                                   # [B,T,L]
    el_full = np.concatenate([el_full, np.zeros((B, 1, L), np.float32)], axis=1)
    el32 = (np.arange(T + 1)[None, :] >= lens[:, None]).astype(np.float32)  # [B,T+1]

    bf = ml_dtypes.bfloat16
    Ew = np.exp(trans)                    # [L,L], rows=from, cols=to
    gsl = [(0, 43), (43, 86), (86, 128)]  # local seq ranges per label group
    el_cores, u0_cores = [], []
    for core in range(NCORES):
        b0 = core * BPC
        E = el_full[b0 : b0 + BPC]          # [128, T+1, L]
        E32 = el32[b0 : b0 + BPC]           # [128, T+1]
        packed = np.zeros((NPART, SP - 2, COLS), np.float32)
        u0 = np.zeros((NPART, COLS), np.float32)
        for c in range(C):
            t0 = c * S
            sl = E[:, t0 + 2 : t0 + SP, :]     # [128, SP-2, L]
            sl32 = E32[:, t0 + 2 : t0 + SP]    # [128, SP-2]
            # tick-1 state u1 = el_{t0+1} * (Wf^T start), start = el_0 for
            # chain 0 / uniform 1 for cold chains; sink1 via el32 gate.
            if c == 0:
                start = E[:, 0, :]                      # [128, L]
            else:
                start = np.ones((BPC, L), np.float32)
            u1 = E[:, t0 + 1, :] * (start @ Ew)         # [128, L]
            sink1 = E32[:, t0 + 1] * start.sum(1)       # [128]
            for g, (s0, s1) in enumerate(gsl):
                nc_ = s1 - s0
                cc = NCOL * c
                packed[32 * g : 32 * g + 32, :, cc : cc + nc_] = sl[s0:s1].transpose(2, 1, 0)
                packed[NACT + g, :, cc : cc + nc_] = sl32[s0:s1].T
                u0[32 * g : 32 * g + 32, cc : cc + nc_] = u1[s0:s1].T
                u0[NACT + g, cc : cc + nc_] = sink1[s0:s1]
        el_cores.append(packed.astype(bf))
        u0_cores.append(u0.astype(bf))

    # ---- stationary operator: block-diag exp(trans) + sink + colsum ----
    Ew = np.exp(trans).astype(np.float32)
    Wf = np.zeros((NPART, MOUT), np.float32)
    for g in range(3):
        a, sk, cs = 32 * g, NACT + g, NPART + g
        Wf[a : a + 32, a : a + 32] = Ew
        Wf[a : a + 32, sk] = 1.0
        Wf[sk, sk] = 1.0
        Wf[a : a + 32, cs] = 1.0
        Wf[sk, cs] = 1.0
    return gp, lens, el_cores, u0_cores, Wf.astype(bf)


def _log(msg):
    import time as _t

    print(f"[kernel {_t.strftime('%H:%M:%S')}] {msg}", flush=True)


def kernel(logits, trans, labels, seq_lens):
    global last_result
    from concourse.bass_utils import run_bass_kernel_spmd

    _log("host prep start")
    gp, lens, el_cores, u0_cores, Wf = _host_prep(logits, trans, labels, seq_lens)
    u0_cores_g = u0_cores
    _log("host prep done")

    if "nc" not in _prog_cache:
        _prog_cache["nc"] = _build_program()
        _log("program built")
    nc = _prog_cache["nc"]

    in_maps = [
        {"el": el_cores[i], "u0": u0_cores[i], "wf": Wf}
        for i in range(NCORES)
    ]
    r = run_bass_kernel_spmd(nc, in_maps, core_ids=list(range(NCORES)))
    last_result = r
    _log("device run done")

    # ---- unshard: column sums of the saved states give both the scale
    # ratios and the captured Z (sink carries Z with actives dead, so the
    # column sum at each boundary tick is the single number needed) ----
    gsl = [(0, 43), (43, 86), (86, 128)]
    colE = np.zeros((C, B), np.float64)   # colsum at t = c*S + BURN - 1
    colF = np.zeros((C, B), np.float64)   # colsum at t = c*S + SP - 1 (= Z_j)
    for core in range(NCORES):
        uE = np.asarray(u0_cores_g[core], np.float64)                 # [99,COLS]
        uF = np.asarray(last_result.results[core]["uF"], np.float64)
        b0 = core * BPC
        for g, (s0, s1) in enumerate(gsl):
            nc_ = s1 - s0
            sE = uE[32 * g : 32 * g + 32].sum(0) + uE[NACT + g]
            sF = uF[32 * g : 32 * g + 32].sum(0) + uF[NACT + g]
            colE[:, b0 + s0 : b0 + s1] = sE.reshape(C, NCOL)[:, :nc_]
            colF[:, b0 + s0 : b0 + s1] = sF.reshape(C, NCOL)[:, :nc_]

    # ---- stitch scales: chain c valid for len in (c*S+BURN, c*S+SP] ----
    j = np.zeros(B, np.int64)
    for c in range(1, C):
        j[lens > c * S + BURN] = c
    with np.errstate(divide="ignore", invalid="ignore"):
        log_rho = np.log(colE[1:]) - np.log(colF[:-1])        # [C-1, B]
        log_gamma = np.concatenate(
            [np.zeros((1, B)), np.cumsum(log_rho, axis=0)], axis=0
        )                                                      # [C, B]
        log_sink = np.log(colF[j, np.arange(B)])
    logZ = log_sink - log_gamma[j, np.arange(B)] + CSHIFT * lens
    return (gp - logZ).astype(np.float32)


# revision 16
# speedup vs baseline: 1.7963x; 1.0179x over previous
"""CRF log-likelihood kernel for Trainium2 (Bass/Tile), 8-core data parallel.

out[b] = gold_path_score(b) - logZ(b)

logZ via chunked cold-start forward chains (exp domain).  The transfer
operator D_{el_t} Wf^T is strictly positive, so it contracts directions at
~0.3/step (Birkhoff): a chain started from the uniform vector forgets its
start after ~10 steps.  Split t = 1..T into C chunks of S steps; chain c
starts cold at t = c*S and runs BURN warm-up ticks + S valid ticks, all C
chains advancing in lockstep columns of the same matmul.  Serial depth drops
from T to SP = BURN + S ticks.

Scale recovery: chain c's state is gamma_c * alpha_t (unknown scalar
gamma_c).  At boundary time c*S + SP - 1 both chain c (final tick) and chain
c+1 (tick BURN) hold the same alpha direction, so the ratio of their Wf
column sums gives gamma_{c+1}/gamma_c exactly.  Sink rows capture
sum(alpha_{len-1}) at t == len per sequence (el32 gating), persist to the
chain end, and are read from the final tick's PSUM.  Host stitches:
logZ = log(sink_j) - log(gamma_j) + CSHIFT*len,  j = chunk containing len.

Per-core layout (128 sequences): partitions 0..95 = 3 label groups x 32,
96..98 = per-group sink rows; psum rows 99..101 = per-group column sums.
Columns: chain c occupies cols [43c, 43c+43); within a column, the 3 label
groups hold 3 different sequences (43+43+42+pad = 128).  Two column groups
of 10 chains each give two independent MM->TT streams that overlap on
PE/DVE.  Emissions exp'd on host, shipped bf16 (4x less DMA than f32
logits; el rounding is ~0.4% -> ~0.1 absolute logZ noise over 512 steps).
"""

import numpy as np
import ml_dtypes

B, T, L = 1024, 512, 32
NCORES = 8
BPC = B // NCORES        # 128 sequences per core
NCOL = 43                # columns per chain (3 label groups: 43+43+42+pad)
NACT = 96                # active label partitions
NPART = 99               # + 3 sink rows
MOUT = 102               # + 3 colsum rows
CSHIFT = 4.5
C = 30                   # chains (chunks)
BURN = 2                 # cold-start warm-up ticks
S = (T - BURN) // C      # valid ticks per chain (17)
assert BURN + C * S == T
SP = BURN + S            # ticks per chain (19)
COLS = NCOL * C          # 1290 total columns
GCOLS = 10 * NCOL        # three groups of 10 chains (430 cols)

_prog_cache = {}
last_result = None       # BassKernelResults of the most recent run (for test.py)


def _build_program():
    import concourse.bacc as bacc
    import concourse.tile as tile
    from concourse import mybir

    f32 = mybir.dt.float32
    bf16 = mybir.dt.bfloat16
    AF = mybir.ActivationFunctionType

    nc = bacc.Bacc("TRN2", target_bir_lowering=False, debug=False, num_devices=NCORES)
    # el slice 0 carries the host-computed tick-2 state u2; slices 1..SP-3
    # carry emissions for ticks 3..SP-1.
    el_d = nc.dram_tensor("el", [NPART, SP - 2, COLS], bf16, kind="ExternalInput")
    wf_d = nc.dram_tensor("wf", [NPART, MOUT], bf16, kind="ExternalInput")
    uF_d = nc.dram_tensor("uF", [NPART, COLS], bf16, kind="ExternalOutput")

    with tile.TileContext(nc) as tc:
        with (
            tc.tile_pool(name="consts", bufs=1) as consts,
            tc.tile_pool(name="elp", bufs=1) as elp,
            tc.tile_pool(name="uA", bufs=3) as uApool,
            tc.tile_pool(name="uB", bufs=3) as uBpool,
            tc.tile_pool(name="uC", bufs=3) as uCpool,
            tc.tile_pool(name="fin", bufs=1) as fin,
            tc.tile_pool(name="psA", bufs=2, space="PSUM") as psApool,
            tc.tile_pool(name="psB", bufs=2, space="PSUM") as psBpool,
            tc.tile_pool(name="psC", bufs=2, space="PSUM") as psCpool,
        ):
            wf_sb = consts.tile([NPART, MOUT], bf16)
            el_sb = elp.tile([NPART, SP - 2, COLS], bf16)

            nc.sync.dma_start(out=wf_sb[:], in_=wf_d[:])
            # el streams in tick-major chunks on the sync queue (the one
            # with the full DMA engine fan-out); first chunks small so the
            # chain starts early.
            edges = [0, 1, 2, 3, 4, 5, 6, 8, 10, 13, SP - 2]
            for k0, k1 in zip(edges[:-1], edges[1:]):
                nc.sync.dma_start(
                    out=el_sb[:, k0:k1, :], in_=el_d[:, k0:k1, :]
                )

            groups = [
                (uApool, psApool, 0, GCOLS),
                (uBpool, psBpool, GCOLS, 2 * GCOLS),
                (uCpool, psCpool, 2 * GCOLS, COLS),
            ]
            # ticks 2..SP-1 (the host precomputes the tick-1 state u1 and
            # its column sums; with BURN=2 the early boundary snapshot IS
            # u1, so the device only captures the final state): captured-Z
            # == column sum at the capture tick, so uF carries everything.
            uprev = [el_sb[:, 0, c0:c1] for (_, _, c0, c1) in groups]
            for k in range(3, SP):
                for gi, (upool, pspool, c0, c1) in enumerate(groups):
                    ps = pspool.tile([MOUT, c1 - c0], f32, tag=f"ps{gi}")
                    nc.tensor.matmul(ps[:], wf_sb[:], uprev[gi], start=True, stop=True)
                    un = upool.tile([NPART, c1 - c0], bf16, tag=f"u{gi}")
                    nc.vector.tensor_mul(
                        un[:], ps[0:NPART, :], el_sb[:, k - 2, c0:c1]
                    )
                    uprev[gi] = un[:]
                    if k == SP - 1:
                        # one output DMA per group on separate queues, each
                        # issued as soon as that group's last TT lands.
                        [nc.gpsimd, nc.scalar, nc.sync][gi].dma_start(
                            out=uF_d[:, c0:c1], in_=un[:]
                        )

    nc.compile()
    return nc


def _host_prep(logits, trans, labels, seq_lens):
    logits = np.ascontiguousarray(np.asarray(logits), dtype=np.float32)
    trans = np.asarray(trans, dtype=np.float32)
    labels = np.asarray(labels)
    lens = np.clip(np.asarray(seq_lens), 1, T).astype(np.int64)

    # ---- gold path score (host: index gathers over small inputs) ----
    tmask = np.arange(T)[None, :] < lens[:, None]
    unary = np.take_along_axis(logits, labels[..., None].astype(np.int64), axis=2)[..., 0]
    gp = (unary * tmask).sum(1) + (trans[labels[:, :-1], labels[:, 1:]] * tmask[:, 1:]).sum(1)

    # ---- emissions: exp on host, masked past seq end; slice t=T is
    # capture-only (el=0 everywhere, el32=1) ----
    lgx = logits.copy()
    lgx[~tmask] = -np.inf
    el_full = np.exp(lgx - CSHIFT)                                   # [B,T,L]
    el_full = np.concatenate([el_full, np.zeros((B, 1, L), np.float32)], axis=1)
    el32 = (np.arange(T + 1)[None, :] >= lens[:, None]).astype(np.float32)  # [B,T+1]

    bf = ml_dtypes.bfloat16
    Ew = np.exp(trans)                    # [L,L], rows=from, cols=to
    gsl = [(0, 43), (43, 86), (86, 128)]  # local seq ranges per label group
    el_cores, u0_cores = [], []
    for core in range(NCORES):
        b0 = core * BPC
        E = el_full[b0 : b0 + BPC]          # [128, T+1, L]
        E32 = el32[b0 : b0 + BPC]           # [128, T+1]
        packed = np.zeros((NPART, SP - 2, COLS), np.float32)
        u1p = np.zeros((NPART, COLS), np.float32)
        for c in range(C):
            t0 = c * S
            sl = E[:, t0 + 3 : t0 + SP, :]     # [128, SP-3, L]
            sl32 = E32[:, t0 + 3 : t0 + SP]    # [128, SP-3]
            # tick-1 state u1 = el_{t0+1} * (Wf^T start), start = el_0 for
            # chain 0 / uniform 1 for cold chains; sink via el32 gate.  u1
            # anchors the boundary column sums (colE); its bf16 rounding,
            # then one more host step, gives the device input u2.
            if c == 0:
                start = E[:, 0, :]                      # [128, L]
            else:
                start = np.ones((BPC, L), np.float32)
            u1 = E[:, t0 + 1, :] * (start @ Ew)         # [128, L]
            sink1 = E32[:, t0 + 1] * start.sum(1)       # [128]
            u1 = u1.astype(bf).astype(np.float32)
            sink1 = sink1.astype(bf).astype(np.float32)
            u2 = E[:, t0 + 2, :] * (u1 @ Ew)            # [128, L]
            sink2 = E32[:, t0 + 2] * (u1.sum(1) + sink1)
            for g, (s0, s1) in enumerate(gsl):
                nc_ = s1 - s0
                cc = NCOL * c
                packed[32 * g : 32 * g + 32, 1:, cc : cc + nc_] = sl[s0:s1].transpose(2, 1, 0)
                packed[NACT + g, 1:, cc : cc + nc_] = sl32[s0:s1].T
                packed[32 * g : 32 * g + 32, 0, cc : cc + nc_] = u2[s0:s1].T
                packed[NACT + g, 0, cc : cc + nc_] = sink2[s0:s1]
                u1p[32 * g : 32 * g + 32, cc : cc + nc_] = u1[s0:s1].T
                u1p[NACT + g, cc : cc + nc_] = sink1[s0:s1]
        el_cores.append(packed.astype(bf))
        u0_cores.append(u1p.astype(bf))

    # ---- stationary operator: block-diag exp(trans) + sink + colsum ----
    Ew = np.exp(trans).astype(np.float32)
    Wf = np.zeros((NPART, MOUT), np.float32)
    for g in range(3):
        a, sk, cs = 32 * g, NACT + g, NPART + g
        Wf[a : a + 32, a : a + 32] = Ew
        Wf[a : a + 32, sk] = 1.0
        Wf[sk, sk] = 1.0
        Wf[a : a + 32, cs] = 1.0
        Wf[sk, cs] = 1.0
    return gp, lens, el_cores, u0_cores, Wf.astype(bf)


def _log(msg):
    import time as _t

    print(f"[kernel {_t.strftime('%H:%M:%S')}] {msg}", flush=True)


def kernel(logits, trans, labels, seq_lens):
    global last_result
    from concourse.bass_utils import run_bass_kernel_spmd

    _log("host prep start")
    gp, lens, el_cores, u0_cores, Wf = _host_prep(logits, trans, labels, seq_lens)
    u0_cores_g = u0_cores
    _log("host prep done")

    if "nc" not in _prog_cache:
        _prog_cache["nc"] = _build_program()
        _log("program built")
    nc = _prog_cache["nc"]

    in_maps = [
        {"el": el_cores[i], "wf": Wf}
        for i in range(NCORES)
    ]
    r = run_bass_kernel_spmd(nc, in_maps, core_ids=list(range(NCORES)))
    last_result = r
    _log("device run done")

    # ---- unshard: column sums of the saved states give both the scale
    # ratios and the captured Z (sink carries Z with actives dead, so the
    # column sum at each boundary tick is the single number needed) ----
    gsl = [(0, 43), (43, 86), (86, 128)]
    colE = np.zeros((C, B), np.float64)   # colsum at t = c*S + BURN - 1
    colF = np.zeros((C, B), np.float64)   # colsum at t = c*S + SP - 1 (= Z_j)
    for core in range(NCORES):
        uE = np.asarray(u0_cores_g[core], np.float64)                 # [99,COLS]
        uF = np.asarray(last_result.results[core]["uF"], np.float64)
        b0 = core * BPC
        for g, (s0, s1) in enumerate(gsl):
            nc_ = s1 - s0
            sE = uE[32 * g : 32 * g + 32].sum(0) + uE[NACT + g]
            sF = uF[32 * g : 32 * g + 32].sum(0) + uF[NACT + g]
            colE[:, b0 + s0 : b0 + s1] = sE.reshape(C, NCOL)[:, :nc_]
            colF[:, b0 + s0 : b0 + s1] = sF.reshape(C, NCOL)[:, :nc_]

    # ---- stitch scales: chain c valid for len in (c*S+BURN, c*S+SP] ----
    j = np.zeros(B, np.int64)
    for c in range(1, C):
        j[lens > c * S + BURN] = c
    with np.errstate(divide="ignore", invalid="ignore"):
        log_rho = np.log(colE[1:]) - np.log(colF[:-1])        # [C-1, B]
        log_gamma = np.concatenate(
            [np.zeros((1, B)), np.cumsum(log_rho, axis=0)], axis=0
        )                                                      # [C, B]
        log_sink = np.log(colF[j, np.arange(B)])
    logZ = log_sink - log_gamma[j, np.arange(B)] + CSHIFT * lens
    return (gp - logZ).astype(np.float32)


# revision 17
# speedup vs baseline: 1.8897x; 1.0520x over previous
"""CRF log-likelihood kernel for Trainium2 (Bass/Tile), 8-core data parallel.

out[b] = gold_path_score(b) - logZ(b)

logZ via chunked cold-start forward chains (exp domain).  The transfer
operator D_{el_t} Wf^T is strictly positive, so it contracts directions at
~0.3/step (Birkhoff): a chain started from the uniform vector forgets its
start after ~10 steps.  Split t = 1..T into C chunks of S steps; chain c
starts cold at t = c*S and runs BURN warm-up ticks + S valid ticks, all C
chains advancing in lockstep columns of the same matmul.  Serial depth drops
from T to SP = BURN + S ticks.

Scale recovery: chain c's state is gamma_c * alpha_t (unknown scalar
gamma_c).  At boundary time c*S + SP - 1 both chain c (final tick) and chain
c+1 (tick BURN) hold the same alpha direction, so the ratio of their Wf
column sums gives gamma_{c+1}/gamma_c exactly.  Sink rows capture
sum(alpha_{len-1}) at t == len per sequence (el32 gating), persist to the
chain end, and are read from the final tick's PSUM.  Host stitches:
logZ = log(sink_j) - log(gamma_j) + CSHIFT*len,  j = chunk containing len.

Per-core layout (128 sequences): partitions 0..95 = 3 label groups x 32,
96..98 = per-group sink rows; psum rows 99..101 = per-group column sums.
Columns: chain c occupies cols [43c, 43c+43); within a column, the 3 label
groups hold 3 different sequences (43+43+42+pad = 128).  Two column groups
of 10 chains each give two independent MM->TT streams that overlap on
PE/DVE.  Emissions exp'd on host, shipped bf16 (4x less DMA than f32
logits; el rounding is ~0.4% -> ~0.1 absolute logZ noise over 512 steps).
"""

import numpy as np
import ml_dtypes

B, T, L = 1024, 512, 32
NCORES = 8
BPC = B // NCORES        # 128 sequences per core
NCOL = 43                # columns per chain (3 label groups: 43+43+42+pad)
NACT = 96                # active label partitions
NPART = 99               # + 3 sink rows
MOUT = 102               # + 3 colsum rows
CSHIFT = 4.5
C = 30                   # chains (chunks)
BURN = 2                 # cold-start warm-up ticks
S = (T - BURN) // C      # valid ticks per chain (17)
assert BURN + C * S == T
SP = BURN + S            # ticks per chain (19)
COLS = NCOL * C          # 1290 total columns
GCOLS = 10 * NCOL        # three groups of 10 chains (430 cols)

_prog_cache = {}
last_result = None       # BassKernelResults of the most recent run (for test.py)


def _build_program():
    import concourse.bacc as bacc
    import concourse.tile as tile
    from concourse import mybir

    f32 = mybir.dt.float32
    bf16 = mybir.dt.bfloat16
    AF = mybir.ActivationFunctionType

    nc = bacc.Bacc("TRN2", target_bir_lowering=False, debug=False, num_devices=NCORES)
    # el slice 0 carries the host-computed tick-3 state u3; slices 1..SP-4
    # carry emissions for ticks 4..SP-1.
    el_d = nc.dram_tensor("el", [NPART, SP - 3, COLS], bf16, kind="ExternalInput")
    wf_d = nc.dram_tensor("wf", [NPART, MOUT], bf16, kind="ExternalInput")
    uF_d = nc.dram_tensor("uF", [NPART, COLS], bf16, kind="ExternalOutput")

    with tile.TileContext(nc) as tc:
        with (
            tc.tile_pool(name="consts", bufs=1) as consts,
            tc.tile_pool(name="elp", bufs=1) as elp,
            tc.tile_pool(name="uA", bufs=3) as uApool,
            tc.tile_pool(name="uB", bufs=3) as uBpool,
            tc.tile_pool(name="uC", bufs=3) as uCpool,
            tc.tile_pool(name="fin", bufs=1) as fin,
            tc.tile_pool(name="psA", bufs=2, space="PSUM") as psApool,
            tc.tile_pool(name="psB", bufs=2, space="PSUM") as psBpool,
            tc.tile_pool(name="psC", bufs=2, space="PSUM") as psCpool,
        ):
            wf_sb = consts.tile([NPART, MOUT], bf16)
            el_sb = elp.tile([NPART, SP - 3, COLS], bf16)

            nc.sync.dma_start(out=wf_sb[:], in_=wf_d[:])
            # el streams in tick-major chunks on the sync queue (the one
            # with the full DMA engine fan-out); first chunks small so the
            # chain starts early.
            edges = [0, 1, 2, 3, 4, 5, 6, 8, 10, 13, SP - 3]
            for k0, k1 in zip(edges[:-1], edges[1:]):
                nc.sync.dma_start(
                    out=el_sb[:, k0:k1, :], in_=el_d[:, k0:k1, :]
                )

            groups = [
                (uApool, psApool, 0, GCOLS),
                (uBpool, psBpool, GCOLS, 2 * GCOLS),
                (uCpool, psCpool, 2 * GCOLS, COLS),
            ]
            # ticks 2..SP-1 (the host precomputes the tick-1 state u1 and
            # its column sums; with BURN=2 the early boundary snapshot IS
            # u1, so the device only captures the final state): captured-Z
            # == column sum at the capture tick, so uF carries everything.
            uprev = [el_sb[:, 0, c0:c1] for (_, _, c0, c1) in groups]
            for k in range(4, SP):
                for gi, (upool, pspool, c0, c1) in enumerate(groups):
                    ps = pspool.tile([MOUT, c1 - c0], f32, tag=f"ps{gi}")
                    nc.tensor.matmul(ps[:], wf_sb[:], uprev[gi], start=True, stop=True)
                    un = upool.tile([NPART, c1 - c0], bf16, tag=f"u{gi}")
                    nc.vector.tensor_mul(
                        un[:], ps[0:NPART, :], el_sb[:, k - 3, c0:c1]
                    )
                    uprev[gi] = un[:]
                    if k == SP - 1:
                        # one output DMA per group on separate queues, each
                        # issued as soon as that group's last TT lands.
                        [nc.gpsimd, nc.scalar, nc.sync][gi].dma_start(
                            out=uF_d[:, c0:c1], in_=un[:]
                        )

    nc.compile()
    return nc


def _host_prep(logits, trans, labels, seq_lens):
    logits = np.ascontiguousarray(np.asarray(logits), dtype=np.float32)
    trans = np.asarray(trans, dtype=np.float32)
    labels = np.asarray(labels)
    lens = np.clip(np.asarray(seq_lens), 1, T).astype(np.int64)

    # ---- gold path score (host: index gathers over small inputs) ----
    tmask = np.arange(T)[None, :] < lens[:, None]
    unary = np.take_along_axis(logits, labels[..., None].astype(np.int64), axis=2)[..., 0]
    gp = (unary * tmask).sum(1) + (trans[labels[:, :-1], labels[:, 1:]] * tmask[:, 1:]).sum(1)

    # ---- emissions: exp on host, masked past seq end; slice t=T is
    # capture-only (el=0 everywhere, el32=1) ----
    lgx = logits.copy()
    lgx[~tmask] = -np.inf
    el_full = np.exp(lgx - CSHIFT)                                   # [B,T,L]
    el_full = np.concatenate([el_full, np.zeros((B, 1, L), np.float32)], axis=1)
    el32 = (np.arange(T + 1)[None, :] >= lens[:, None]).astype(np.float32)  # [B,T+1]

    bf = ml_dtypes.bfloat16
    Ew = np.exp(trans)                    # [L,L], rows=from, cols=to
    gsl = [(0, 43), (43, 86), (86, 128)]  # local seq ranges per label group
    el_cores, u0_cores = [], []
    for core in range(NCORES):
        b0 = core * BPC
        E = el_full[b0 : b0 + BPC]          # [128, T+1, L]
        E32 = el32[b0 : b0 + BPC]           # [128, T+1]
        packed = np.zeros((NPART, SP - 3, COLS), np.float32)
        u1p = np.zeros((NPART, COLS), np.float32)
        for c in range(C):
            t0 = c * S
            sl = E[:, t0 + 4 : t0 + SP, :]     # [128, SP-4, L]
            sl32 = E32[:, t0 + 4 : t0 + SP]    # [128, SP-4]
            # tick-1 state u1 = el_{t0+1} * (Wf^T start), start = el_0 for
            # chain 0 / uniform 1 for cold chains; sink via el32 gate.  u1
            # anchors the boundary column sums (colE); its bf16 rounding,
            # then one more host step, gives the device input u2.
            if c == 0:
                start = E[:, 0, :]                      # [128, L]
            else:
                start = np.ones((BPC, L), np.float32)
            u1 = E[:, t0 + 1, :] * (start @ Ew)         # [128, L]
            sink1 = E32[:, t0 + 1] * start.sum(1)       # [128]
            u1 = u1.astype(bf).astype(np.float32)
            sink1 = sink1.astype(bf).astype(np.float32)
            u2 = E[:, t0 + 2, :] * (u1 @ Ew)            # [128, L]
            sink2 = E32[:, t0 + 2] * (u1.sum(1) + sink1)
            u2 = u2.astype(bf).astype(np.float32)
            sink2 = sink2.astype(bf).astype(np.float32)
            u3 = E[:, t0 + 3, :] * (u2 @ Ew)            # [128, L]
            sink3 = E32[:, t0 + 3] * (u2.sum(1) + sink2)
            for g, (s0, s1) in enumerate(gsl):
                nc_ = s1 - s0
                cc = NCOL * c
                packed[32 * g : 32 * g + 32, 1:, cc : cc + nc_] = sl[s0:s1].transpose(2, 1, 0)
                packed[NACT + g, 1:, cc : cc + nc_] = sl32[s0:s1].T
                packed[32 * g : 32 * g + 32, 0, cc : cc + nc_] = u3[s0:s1].T
                packed[NACT + g, 0, cc : cc + nc_] = sink3[s0:s1]
                u1p[32 * g : 32 * g + 32, cc : cc + nc_] = u1[s0:s1].T
                u1p[NACT + g, cc : cc + nc_] = sink1[s0:s1]
        el_cores.append(packed.astype(bf))
        u0_cores.append(u1p.astype(bf))

    # ---- stationary operator: block-diag exp(trans) + sink + colsum ----
    Ew = np.exp(trans).astype(np.float32)
    Wf = np.zeros((NPART, MOUT), np.float32)
    for g in range(3):
        a, sk, cs = 32 * g, NACT + g, NPART + g
        Wf[a : a + 32, a : a + 32] = Ew
        Wf[a : a + 32, sk] = 1.0
        Wf[sk, sk] = 1.0
        Wf[a : a + 32, cs] = 1.0
        Wf[sk, cs] = 1.0
    return gp, lens, el_cores, u0_cores, Wf.astype(bf)


def _log(msg):
    import time as _t

    print(f"[kernel {_t.strftime('%H:%M:%S')}] {msg}", flush=True)


def kernel(logits, trans, labels, seq_lens):
    global last_result
    from concourse.bass_utils import run_bass_kernel_spmd

    _log("host prep start")
    gp, lens, el_cores, u0_cores, Wf = _host_prep(logits, trans, labels, seq_lens)
    u0_cores_g = u0_cores
    _log("host prep done")

    if "nc" not in _prog_cache:
        _prog_cache["nc"] = _build_program()
        _log("program built")
    nc = _prog_cache["nc"]

    in_maps = [
        {"el": el_cores[i], "wf": Wf}
        for i in range(NCORES)
    ]
    r = run_bass_kernel_spmd(nc, in_maps, core_ids=list(range(NCORES)))
    last_result = r
    _log("device run done")

    # ---- unshard: column sums of the saved states give both the scale
    # ratios and the captured Z (sink carries Z with actives dead, so the
    # column sum at each boundary tick is the single number needed) ----
    gsl = [(0, 43), (43, 86), (86, 128)]
    colE = np.zeros((C, B), np.float64)   # colsum at t = c*S + BURN - 1
    colF = np.zeros((C, B), np.float64)   # colsum at t = c*S + SP - 1 (= Z_j)
    for core in range(NCORES):
        uE = np.asarray(u0_cores_g[core], np.float64)                 # [99,COLS]
        uF = np.asarray(last_result.results[core]["uF"], np.float64)
        b0 = core * BPC
        for g, (s0, s1) in enumerate(gsl):
            nc_ = s1 - s0
            sE = uE[32 * g : 32 * g + 32].sum(0) + uE[NACT + g]
            sF = uF[32 * g : 32 * g + 32].sum(0) + uF[NACT + g]
            colE[:, b0 + s0 : b0 + s1] = sE.reshape(C, NCOL)[:, :nc_]
            colF[:, b0 + s0 : b0 + s1] = sF.reshape(C, NCOL)[:, :nc_]

    # ---- stitch scales: chain c valid for len in (c*S+BURN, c*S+SP] ----
    j = np.zeros(B, np.int64)
    for c in range(1, C):
        j[lens > c * S + BURN] = c
    with np.errstate(divide="ignore", invalid="ignore"):
        log_rho = np.log(colE[1:]) - np.log(colF[:-1])        # [C-1, B]
        log_gamma = np.concatenate(
            [np.zeros((1, B)), np.cumsum(log_rho, axis=0)], axis=0
        )                                                      # [C, B]
        log_sink = np.log(colF[j, np.arange(B)])
    logZ = log_sink - log_gamma[j, np.arange(B)] + CSHIFT * lens
    return (gp - logZ).astype(np.float32)
